# revision 23
# baseline (speedup 1.0000x reference)
"""Bahdanau attention on 8 Trainium2 NeuronCores.

Reference computation (B=32, S=2048, H=1024, fp32):
    q_proj = query @ Wa_w.T + Wa_b            # [B,1,H]
    k_proj = keys @ Ua_w.T + Ua_b             # [B,S,H]
    e      = tanh(q_proj + k_proj)            # [B,S,H]
    scores = e @ Va_w.T + Va_b                # [B,S,1] -> [B,1,S]
    weights = softmax(scores, axis=-1)
    context = weights @ keys                  # [B,1,H]
    returns (context, weights)

Sharding: data-parallel over batch, 4 batches per core; the small
Wa/Ua/Va weights are replicated.

Per-core dataflow (all matmuls contract over the partition dim):
  - keys are uploaded pre-transposed per batch as keysT[h, s] (bf16), so the
    big k_proj matmul runs with Ua_w.T chunks stationary and keysT streaming:
    out[o_tile(128p), s_chunk(512)] accumulates over 8 h-tiles in PSUM.
  - tanh(k_proj + q_proj[o] + Wa_b[o] + Ua_b[o]) is one ScalarE activation per
    tile: the whole per-o bias vector is the per-partition activation bias.
  - scores = Va . e is 8 accumulating matmuls with Va chunk stationary (M=1).
  - softmax skips the max-subtraction (scores are bounded by sum|Va| ~ 16, so
    exp cannot overflow in fp32; softmax is shift-invariant so the result is
    identical to the reference up to rounding).  exp runs on ScalarE with
    accum_out giving the denominator chunks for free.
  - context accumulates per chunk with UNNORMALIZED p = exp(scores): p is
    replicated across partitions by a ones-vector matmul, then
    sum_s p[s] * keysT[h, s] runs on VectorE as fused scalar_tensor_tensor
    ops against the SBUF-resident keysT.  The 1/sum(p) normalization is
    applied once at the end (context scale + weights output), so the context
    reduction overlaps the remaining k_proj matmuls instead of serializing
    after the whole batch.
"""

import numpy as np
import ml_dtypes

import concourse.bass as bass
import concourse.mybir as mybir
import concourse.tile as tile
import concourse.bass_isa as bass_isa
from concourse import bacc
from concourse import bass_utils

BF16 = mybir.dt.bfloat16
F32 = mybir.dt.float32
AF = mybir.ActivationFunctionType
ALU = mybir.AluOpType

N_CORES = 8
B, S, H = 32, 2048, 1024
BPC = B // N_CORES          # batches per core
HT = H // 128               # h (and o) tiles of 128
SC = S // 512               # s chunks of 512

_cache = {}


def build_module():
    nc = bacc.Bacc("TRN2", target_bir_lowering=False, debug=False,
                   enable_asserts=False, num_devices=N_CORES)

    # Per-core inputs
    keysT_d = nc.dram_tensor("keysT", [BPC, SC, 128, HT * 512], BF16, kind="ExternalInput").ap()
    qT_d = nc.dram_tensor("qT", [HT, 128, BPC], BF16, kind="ExternalInput").ap()
    # Replicated weights
    uaT_d = nc.dram_tensor("uaT", [HT, 128, H], BF16, kind="ExternalInput").ap()  # [ot, p, ht*128+o_in]
    waT_d = nc.dram_tensor("waT", [HT, 128, H], BF16, kind="ExternalInput").ap()
    vaT_d = nc.dram_tensor("vaT", [HT, 128], F32, kind="ExternalInput").ap()
    wbT_d = nc.dram_tensor("wbT", [128, HT], F32, kind="ExternalInput").ap()
    ubT_d = nc.dram_tensor("ubT", [128, HT], F32, kind="ExternalInput").ap()
    ones_d = nc.dram_tensor("ones", [1, 128], BF16, kind="ExternalInput").ap()
    ones32_d = nc.dram_tensor("ones32", [1, 128], F32, kind="ExternalInput").ap()
    # Outputs
    wout_d = nc.dram_tensor("wout", [BPC, S], F32, kind="ExternalOutput").ap()
    ctxout_d = nc.dram_tensor("ctxout", [BPC, 128, HT], F32, kind="ExternalOutput").ap()

    with tile.TileContext(nc) as tc:
        with (
            tc.tile_pool(name="const", bufs=1) as cpool,
            tc.tile_pool(name="keys", bufs=2) as kpool,
            tc.tile_pool(name="work", bufs=2) as wpool,
            tc.tile_pool(name="kp_ps", bufs=6, space="PSUM") as kp_ps,
            tc.tile_pool(name="misc_ps", bufs=2, space="PSUM") as misc_ps,
        ):
            # ---- constants into SBUF ----
            # DMA order is tuned so PE never waits: qp inputs (q, wa piece 0)
            # first, then ua piece 0 + keys0 chunk 0 (first kp group), then the
            # remaining wa/ua pieces interleaved, with keys0 chunks spread in
            # between at the rate the kp groups consume them.
            q_sb = cpool.tile([128, HT * BPC], BF16, tag="q")
            nc.sync.dma_start(out=q_sb[:], in_=qT_d.rearrange("t p b -> p t b"))
            wa_sb = cpool.tile([128, HT * H], BF16, tag="wa")
            nc.sync.dma_start(out=wa_sb[:, 0:H], in_=waT_d[0])
            ua_sb = cpool.tile([128, HT * H], BF16, tag="ua")
            nc.sync.dma_start(out=ua_sb[:, 0:H], in_=uaT_d[0])

            keys_sb = {}
            kt0 = kpool.tile([128, SC * HT * 512], BF16, tag="keys", name="keys0")
            keys_sb[0] = kt0
            nc.sync.dma_start(out=kt0[:, 0:HT * 512], in_=keysT_d[0, 0])

            va_sb = cpool.tile([128, HT], F32, tag="va")
            nc.sync.dma_start(out=va_sb[:], in_=vaT_d.rearrange("t p -> p t"))
            va16_sb = cpool.tile([128, HT], BF16, tag="va16")
            nc.vector.tensor_copy(va16_sb[:], va_sb[:])
            ones_sb = cpool.tile([1, 128], BF16, tag="ones")
            nc.sync.dma_start(out=ones_sb[:], in_=ones_d)
            ones32_sb = cpool.tile([1, 128], F32, tag="ones32")
            nc.sync.dma_start(out=ones32_sb[:], in_=ones32_d)
            wb_sb = cpool.tile([128, HT], F32, tag="wb")
            nc.sync.dma_start(out=wb_sb[:], in_=wbT_d)
            ub_sb = cpool.tile([128, HT], F32, tag="ub")
            nc.sync.dma_start(out=ub_sb[:], in_=ubT_d)
            bias_sb = cpool.tile([128, HT], F32, tag="bias")
            nc.vector.tensor_tensor(out=bias_sb[:], in0=wb_sb[:], in1=ub_sb[:], op=ALU.add)

            def load_keys(b):
                # chunk-major layout: [:, c*HT*512 + ht*512 + s]
                t = kpool.tile([128, SC * HT * 512], BF16, tag="keys", name=f"keys{b}")
                for c in range(SC):
                    nc.sync.dma_start(
                        out=t[:, c * HT * 512:(c + 1) * HT * 512], in_=keysT_d[b, c])
                keys_sb[b] = t

            t0 = kt0
            for ot in range(1, HT):
                nc.sync.dma_start(out=wa_sb[:, ot * H:(ot + 1) * H], in_=waT_d[ot])
                nc.sync.dma_start(out=ua_sb[:, ot * H:(ot + 1) * H], in_=uaT_d[ot])
                if ot in (3, 5, 7):
                    c = {3: 1, 5: 2, 7: 3}[ot]
                    nc.sync.dma_start(
                        out=t0[:, c * HT * 512:(c + 1) * HT * 512], in_=keysT_d[0, c])

            # q_proj group: qp_sb[:, ot*BPC + b] = (Wa_w @ q + Wa_b + Ua_b)[o]
            qp_sb = cpool.tile([128, HT * BPC], F32, tag="qp")

            def qp_group(ot):
                ps = misc_ps.tile([128, 512], F32, tag="mps", name=f"qp_ps{ot}")
                for ht in range(HT):
                    nc.tensor.matmul(
                        ps[:, :BPC],
                        lhsT=wa_sb[:, ot * H + ht * 128: ot * H + ht * 128 + 128],
                        rhs=q_sb[:, ht * BPC:(ht + 1) * BPC],
                        start=(ht == 0), stop=(ht == HT - 1),
                    )
                nc.scalar.activation(
                    qp_sb[:, ot * BPC:(ot + 1) * BPC], ps[:, :BPC],
                    AF.Identity, bias=bias_sb[:, ot:ot + 1],
                )

            # ---- per-batch state ----
            state = {}

            def batch_state(b):
                p32 = wpool.tile([1, S], F32, tag="p32", name=f"p32_{b}")
                pacc = wpool.tile([1, SC], F32, tag="pacc", name=f"pacc_{b}")
                ctx_c = [wpool.tile([128, HT], F32, tag="ctxc", bufs=8, name=f"ctxc_{b}_{c}")
                         for c in range(SC)]
                return p32, pacc, ctx_c

            def kp_group(b, c, ot, eT):
                """one k_proj psum accumulation group + its tanh."""
                ps = kp_ps.tile([128, 512], F32, tag="kp", name=f"kp_ps{b}_{c}_{ot}")
                for ht in range(HT):
                    nc.tensor.matmul(
                        ps[:],
                        lhsT=ua_sb[:, ot * H + ht * 128: ot * H + ht * 128 + 128],
                        rhs=keys_sb[b][:, (c * HT + ht) * 512: (c * HT + ht) * 512 + 512],
                        start=(ht == 0), stop=(ht == HT - 1),
                    )
                nc.scalar.activation(
                    eT[:, ot * 512:(ot + 1) * 512], ps[:],
                    AF.Tanh, bias=qp_sb[:, ot * BPC + b: ot * BPC + b + 1],
                )

            def scores_mms(b, c, eT):
                """scores = Va . e on DVE: chained (eT[ot] * va[ot]) + acc,
                then one GPSIMD all-reduce across the 128 partitions."""
                acc = None
                for ot in range(HT):
                    nacc = wpool.tile([128, 512], F32, tag="sacc", bufs=3,
                                      name=f"sacc_{b}_{c}_{ot}")
                    if acc is None:
                        nc.vector.tensor_scalar_mul(
                            nacc[:], eT[:, ot * 512:(ot + 1) * 512], va_sb[:, ot:ot + 1])
                    else:
                        nc.vector.scalar_tensor_tensor(
                            out=nacc[:], in0=eT[:, ot * 512:(ot + 1) * 512],
                            scalar=va_sb[:, ot:ot + 1], in1=acc[:],
                            op0=ALU.mult, op1=ALU.add)
                    acc = nacc
                sred = wpool.tile([128, 512], F32, tag="sred", bufs=2,
                                  name=f"sred_{b}_{c}")
                nc.gpsimd.partition_all_reduce(sred[:], acc[:], channels=128,
                                               reduce_op=bass_isa.ReduceOp.add)
                return sred

            def exp_block(b, c, sps):
                """p = exp(scores) chunk + running denominator + bf16 copy."""
                p32, pacc, ctx_c = state[b]
                nc.scalar.activation(
                    p32[:, c * 512:(c + 1) * 512], sps[0:1, :],
                    AF.Exp, accum_out=pacc[:, c:c + 1],
                )
                p16 = wpool.tile([1, 512], BF16, tag="p16", bufs=3, name=f"p16_{b}_{c}")
                nc.scalar.copy(p16[:], p32[:, c * 512:(c + 1) * 512])
                return p16

            def ctx_block(b, c, p16):
                """replicate p across partitions, then per-chunk context on DVE."""
                p32, pacc, ctx_c = state[b]
                rep_ps = misc_ps.tile([128, 512], F32, tag="mps", name=f"rep_ps{b}_{c}")
                nc.tensor.matmul(rep_ps[:], lhsT=ones_sb[:], rhs=p16[:])
                prep = wpool.tile([128, 512], BF16, tag="prep", bufs=3, name=f"prep_{b}_{c}")
                nc.vector.tensor_copy(prep[:], rep_ps[:])
                # ctx_c[c][:, ht] = sum_s keysT[ht][:, s] * p[s] over this chunk
                for ht in range(HT):
                    tout = wpool.tile([128, 512], BF16, tag="ttro", name=f"ttro_{b}_{c}_{ht}")
                    nc.vector.scalar_tensor_tensor(
                        out=tout[:],
                        in0=keys_sb[b][:, (c * HT + ht) * 512: (c * HT + ht) * 512 + 512],
                        scalar=1.0,
                        in1=prep[:],
                        op0=ALU.bypass, op1=ALU.mult,
                        accum_out=ctx_c[c][:, ht:ht + 1],
                    )

            def finalize(b):
                """normalize: weights out, context = (sum_c ctx_c) / sum(p)."""
                p32, pacc, ctx_c = state[b]
                l_t = wpool.tile([1, 1], F32, tag="l", name=f"l_{b}")
                nc.vector.tensor_reduce(l_t[:], pacc[:], axis=mybir.AxisListType.X, op=ALU.add)
                r_t = wpool.tile([1, 1], F32, tag="r", name=f"r_{b}")
                nc.vector.reciprocal(r_t[:], l_t[:])
                w_t = wpool.tile([1, S], F32, tag="w", name=f"w_{b}")
                nc.scalar.mul(w_t[:], p32[:], r_t[:])
                nc.sync.dma_start(out=wout_d[b], in_=w_t[:])
                # replicate r across partitions (fp32 K=1 matmul), scale context
                rep2 = misc_ps.tile([128, 512], F32, tag="mps", name=f"rep2_{b}")
                nc.tensor.matmul(rep2[:, 0:1], lhsT=ones32_sb[:], rhs=r_t[:])
                r128 = wpool.tile([128, 1], F32, tag="r128", name=f"r128_{b}")
                nc.vector.tensor_copy(r128[:], rep2[:, 0:1])
                s01 = wpool.tile([128, HT], F32, tag="s01", name=f"s01_{b}")
                nc.vector.tensor_tensor(out=s01[:], in0=ctx_c[0][:], in1=ctx_c[1][:], op=ALU.add)
                s23 = wpool.tile([128, HT], F32, tag="s23", name=f"s23_{b}")
                nc.vector.tensor_tensor(out=s23[:], in0=ctx_c[2][:], in1=ctx_c[3][:], op=ALU.add)
                csum = wpool.tile([128, HT], F32, tag="csum", name=f"csum_{b}")
                nc.vector.tensor_tensor(out=csum[:], in0=s01[:], in1=s23[:], op=ALU.add)
                ctx = wpool.tile([128, HT], F32, tag="ctx", name=f"ctx_{b}")
                nc.vector.tensor_scalar_mul(ctx[:], csum[:], r128[:])
                nc.sync.dma_start(out=ctxout_d[b], in_=ctx[:])

            # ---- main loop ----
            # The previous chunk's scores / exp / context work is interleaved
            # between this chunk's kp groups so that by the time PE reaches
            # each dependent matmul (scores, replicate) its ACT/DVE inputs are
            # already done — keeps the PE stream stall-free.
            pending = None
            for b in range(BPC):
                state[b] = batch_state(b)
                if b >= 1 and b + 1 < BPC:
                    load_keys(b + 1)
                for c in range(SC):
                    eT = wpool.tile([128, HT * 512], BF16, tag="eT", bufs=3,
                                    name=f"eT_{b}_{c}")
                    for ot in range(HT):
                        if b == 0 and c == 0:
                            qp_group(ot)
                        kp_group(b, c, ot, eT)
                        if b == 0 and c == 0 and ot == HT - 1:
                            load_keys(1)
                        if pending is not None:
                            pb, pc, peT, psps, pp16 = pending
                            if ot == 0:
                                psps = scores_mms(pb, pc, peT)
                                pending = (pb, pc, peT, psps, pp16)
                            elif ot == 1:
                                pp16 = exp_block(pb, pc, psps)
                                pending = (pb, pc, peT, psps, pp16)
                            elif ot == 2:
                                ctx_block(pb, pc, pp16)
                            elif ot == 4 and pc == SC - 1:
                                finalize(pb)
                    pending = (b, c, eT, None, None)
            pb, pc, peT, _, _ = pending
            # final chunk: scores on PE (the DVE pipeline has no later kp work
            # to hide behind, and the PE is idle here anyway)
            fps = misc_ps.tile([128, 512], F32, tag="mps", name="final_s_ps")
            for ot in range(HT):
                nc.tensor.matmul(
                    fps[0:1, :],
                    lhsT=va16_sb[:, ot:ot + 1],
                    rhs=peT[:, ot * 512:(ot + 1) * 512],
                    start=(ot == 0), stop=(ot == HT - 1),
                )
            p16 = exp_block(pb, pc, fps)
            ctx_block(pb, pc, p16)
            finalize(pb)

    nc.compile()
    return nc


def _prep_inputs(query, keys, Wa_w, Wa_b, Ua_w, Ua_b, Va_w, Va_b):
    bf16 = ml_dtypes.bfloat16
    # Va_b shifts every score equally; softmax is shift-invariant and scores
    # themselves are not returned, so it never affects the output.
    del Va_b
    uaT = np.ascontiguousarray(
        Ua_w.T.reshape(HT, 128, HT, 128).transpose(2, 1, 0, 3)).reshape(HT, 128, H).astype(bf16)
    waT = np.ascontiguousarray(
        Wa_w.T.reshape(HT, 128, HT, 128).transpose(2, 1, 0, 3)).reshape(HT, 128, H).astype(bf16)
    vaT = Va_w[0].reshape(HT, 128).astype(np.float32)
    wbT = np.ascontiguousarray(Wa_b.reshape(HT, 128).T).astype(np.float32)
    ubT = np.ascontiguousarray(Ua_b.reshape(HT, 128).T).astype(np.float32)
    ones = np.ones((1, 128), dtype=bf16)
    ones32 = np.ones((1, 128), dtype=np.float32)

    in_maps = []
    for core in range(N_CORES):
        bs = slice(core * BPC, (core + 1) * BPC)
        kT = np.ascontiguousarray(
            keys[bs].reshape(BPC, SC, 512, HT, 128).transpose(0, 1, 4, 3, 2)
        ).reshape(BPC, SC, 128, HT * 512).astype(bf16)
        qT = np.ascontiguousarray(
            query[bs, 0, :].T).reshape(HT, 128, BPC).astype(bf16)
        in_maps.append({
            "keysT": kT, "qT": qT, "uaT": uaT, "waT": waT, "vaT": vaT,
            "wbT": wbT, "ubT": ubT, "ones": ones, "ones32": ones32,
        })
    return in_maps


def kernel(query, keys, Wa_w, Wa_b, Ua_w, Ua_b, Va_w, Va_b):
    if "nc" not in _cache:
        _cache["nc"] = build_module()
    nc = _cache["nc"]

    in_maps = _prep_inputs(query, keys, Wa_w, Wa_b, Ua_w, Ua_b, Va_w, Va_b)
    res = bass_utils.run_bass_kernel_spmd(nc, in_maps, core_ids=list(range(N_CORES)))

    context = np.empty((B, 1, H), dtype=np.float32)
    weights = np.empty((B, 1, S), dtype=np.float32)
    for core in range(N_CORES):
        out = res.results[core]
        weights[core * BPC:(core + 1) * BPC, 0, :] = out["wout"]
        # ctxout[b, p, t] holds context[h = t*128 + p]
        context[core * BPC:(core + 1) * BPC, 0, :] = (
            out["ctxout"].transpose(0, 2, 1).reshape(BPC, H))
    return (context, weights)


# revision 24
# speedup vs baseline: 1.1723x; 1.1723x over previous
"""Bahdanau attention on 8 Trainium2 NeuronCores.

Reference computation (B=32, S=2048, H=1024, fp32):
    q_proj = query @ Wa_w.T + Wa_b            # [B,1,H]
    k_proj = keys @ Ua_w.T + Ua_b             # [B,S,H]
    e      = tanh(q_proj + k_proj)            # [B,S,H]
    scores = e @ Va_w.T + Va_b                # [B,S,1] -> [B,1,S]
    weights = softmax(scores, axis=-1)
    context = weights @ keys                  # [B,1,H]
    returns (context, weights)

Sharding: data-parallel over batch, 4 batches per core; the small
Wa/Ua/Va weights are replicated.

Per-core dataflow (all matmuls contract over the partition dim):
  - keys are uploaded pre-transposed per batch as keysT[h, s] (bf16), so the
    big k_proj matmul runs with Ua_w.T chunks stationary and keysT streaming:
    out[o_tile(128p), s_chunk(512)] accumulates over 8 h-tiles in PSUM.
  - tanh(k_proj + q_proj[o] + Wa_b[o] + Ua_b[o]) is one ScalarE activation per
    tile: the whole per-o bias vector is the per-partition activation bias.
  - scores = Va . e is 8 accumulating matmuls with Va chunk stationary (M=1).
  - softmax skips the max-subtraction (scores are bounded by sum|Va| ~ 16, so
    exp cannot overflow in fp32; softmax is shift-invariant so the result is
    identical to the reference up to rounding).  exp runs on ScalarE with
    accum_out giving the denominator chunks for free.
  - context accumulates per chunk with UNNORMALIZED p = exp(scores): p is
    replicated across partitions by a ones-vector matmul, then
    sum_s p[s] * keysT[h, s] runs on VectorE as fused scalar_tensor_tensor
    ops against the SBUF-resident keysT.  The 1/sum(p) normalization is
    applied once at the end (context scale + weights output), so the context
    reduction overlaps the remaining k_proj matmuls instead of serializing
    after the whole batch.
"""

import numpy as np
import ml_dtypes

import concourse.bass as bass
import concourse.mybir as mybir
import concourse.tile as tile
import concourse.bass_isa as bass_isa
from concourse import bacc
from concourse import bass_utils

BF16 = mybir.dt.bfloat16
F32 = mybir.dt.float32
FP8 = mybir.dt.float8e4
KSCALE = 16.0
USCALE = 256.0
INV_SCALE = 1.0 / (KSCALE * USCALE)
AF = mybir.ActivationFunctionType
ALU = mybir.AluOpType

N_CORES = 8
B, S, H = 32, 2048, 1024
BPC = B // N_CORES          # batches per core
HT = H // 128               # h (and o) tiles of 128
SC = S // 512               # s chunks of 512

_cache = {}


def build_module():
    nc = bacc.Bacc("TRN2", target_bir_lowering=False, debug=False,
                   enable_asserts=False, num_devices=N_CORES)

    # Per-core inputs
    keysT_d = nc.dram_tensor("keysT", [BPC, SC, 128, HT * 512], BF16, kind="ExternalInput").ap()
    qT_d = nc.dram_tensor("qT", [HT, 128, BPC], BF16, kind="ExternalInput").ap()
    keys8_d = nc.dram_tensor("keys8", [BPC, SC, 128, HT * 512], FP8, kind="ExternalInput").ap()
    keys8r_d = nc.dram_tensor("keys8r", [BPC, SC, 128, HT * 512], FP8, kind="ExternalInput").ap()
    # Replicated weights ([ot, p, ht*128+o_in] o-major pieces)
    ua8_d = nc.dram_tensor("ua8", [HT, 128, H], FP8, kind="ExternalInput").ap()
    ua8r_d = nc.dram_tensor("ua8r", [HT, 128, H], FP8, kind="ExternalInput").ap()
    waT_d = nc.dram_tensor("waT", [HT, 128, H], BF16, kind="ExternalInput").ap()
    vaT_d = nc.dram_tensor("vaT", [HT, 128], F32, kind="ExternalInput").ap()
    wbT_d = nc.dram_tensor("wbT", [128, HT], F32, kind="ExternalInput").ap()
    ubT_d = nc.dram_tensor("ubT", [128, HT], F32, kind="ExternalInput").ap()
    ones_d = nc.dram_tensor("ones", [1, 128], BF16, kind="ExternalInput").ap()
    ones32_d = nc.dram_tensor("ones32", [1, 128], F32, kind="ExternalInput").ap()
    # Outputs
    wout_d = nc.dram_tensor("wout", [BPC, S], F32, kind="ExternalOutput").ap()
    ctxout_d = nc.dram_tensor("ctxout", [BPC, 128, HT], F32, kind="ExternalOutput").ap()

    with tile.TileContext(nc) as tc:
        with (
            tc.tile_pool(name="const", bufs=1) as cpool,
            tc.tile_pool(name="keys", bufs=2) as kpool,
            tc.tile_pool(name="work", bufs=2) as wpool,
            tc.tile_pool(name="kp_ps", bufs=6, space="PSUM") as kp_ps,
            tc.tile_pool(name="misc_ps", bufs=2, space="PSUM") as misc_ps,
        ):
            # ---- constants into SBUF ----
            # DMA order is tuned so PE never waits: qp inputs (q, wa piece 0)
            # first, then ua piece 0 + keys0 chunk 0 (first kp group), then the
            # remaining wa/ua pieces interleaved, with keys0 chunks spread in
            # between at the rate the kp groups consume them.
            q_sb = cpool.tile([128, HT * BPC], BF16, tag="q")
            nc.sync.dma_start(out=q_sb[:], in_=qT_d.rearrange("t p b -> p t b"))
            wa_sb = cpool.tile([128, HT * H], BF16, tag="wa")
            nc.sync.dma_start(out=wa_sb[:, 0:H], in_=waT_d[0])
            ua8_sb = cpool.tile([128, HT * H], FP8, tag="ua8")
            nc.sync.dma_start(out=ua8_sb[:, 0:H], in_=ua8_d[0])
            ua8r_sb = cpool.tile([128, HT * H], FP8, tag="ua8r")
            nc.sync.dma_start(out=ua8r_sb[:, 0:H], in_=ua8r_d[0])

            keys_sb = {}
            k8_sb = {}

            def load_k8(b, c):
                t8 = wpool.tile([128, HT * 512], FP8, tag="k8", bufs=3,
                                name=f"k8_{b}_{c}")
                nc.sync.dma_start(out=t8[:], in_=keys8_d[b, c])
                t8r = wpool.tile([128, HT * 512], FP8, tag="k8r", bufs=3,
                                 name=f"k8r_{b}_{c}")
                nc.sync.dma_start(out=t8r[:], in_=keys8r_d[b, c])
                k8_sb[(b, c)] = (t8, t8r)

            load_k8(0, 0)

            va_sb = cpool.tile([128, HT], F32, tag="va")
            nc.sync.dma_start(out=va_sb[:], in_=vaT_d.rearrange("t p -> p t"))
            va16_sb = cpool.tile([128, HT], BF16, tag="va16")
            nc.vector.tensor_copy(va16_sb[:], va_sb[:])
            ones_sb = cpool.tile([1, 128], BF16, tag="ones")
            nc.sync.dma_start(out=ones_sb[:], in_=ones_d)
            ones32_sb = cpool.tile([1, 128], F32, tag="ones32")
            nc.sync.dma_start(out=ones32_sb[:], in_=ones32_d)
            wb_sb = cpool.tile([128, HT], F32, tag="wb")
            nc.sync.dma_start(out=wb_sb[:], in_=wbT_d)
            ub_sb = cpool.tile([128, HT], F32, tag="ub")
            nc.sync.dma_start(out=ub_sb[:], in_=ubT_d)
            bias_sb = cpool.tile([128, HT], F32, tag="bias")
            nc.vector.tensor_tensor(out=bias_sb[:], in0=wb_sb[:], in1=ub_sb[:], op=ALU.add)

            def load_keys(b):
                # chunk-major layout: [:, c*HT*512 + ht*512 + s]
                t = kpool.tile([128, SC * HT * 512], BF16, tag="keys", name=f"keys{b}")
                for c in range(SC):
                    nc.sync.dma_start(
                        out=t[:, c * HT * 512:(c + 1) * HT * 512], in_=keysT_d[b, c])
                keys_sb[b] = t

            for ot in range(1, HT):
                nc.sync.dma_start(out=wa_sb[:, ot * H:(ot + 1) * H], in_=waT_d[ot])
                nc.sync.dma_start(out=ua8_sb[:, ot * H:(ot + 1) * H], in_=ua8_d[ot])
                nc.sync.dma_start(out=ua8r_sb[:, ot * H:(ot + 1) * H], in_=ua8r_d[ot])
                if ot == 4:
                    load_k8(0, 1)
            load_keys(0)

            # q_proj group: qp_sb[:, ot*BPC + b] = (Wa_w @ q + Wa_b + Ua_b)[o]
            qp_sb = cpool.tile([128, HT * BPC], F32, tag="qp")

            def qp_group(ot):
                ps = misc_ps.tile([128, 512], F32, tag="mps", name=f"qp_ps{ot}")
                for ht in range(HT):
                    nc.tensor.matmul(
                        ps[:, :BPC],
                        lhsT=wa_sb[:, ot * H + ht * 128: ot * H + ht * 128 + 128],
                        rhs=q_sb[:, ht * BPC:(ht + 1) * BPC],
                        start=(ht == 0), stop=(ht == HT - 1),
                    )
                nc.scalar.activation(
                    qp_sb[:, ot * BPC:(ot + 1) * BPC], ps[:, :BPC],
                    AF.Identity, bias=bias_sb[:, ot:ot + 1],
                )

            # ---- per-batch state ----
            state = {}

            def batch_state(b):
                p32 = wpool.tile([1, S], F32, tag="p32", name=f"p32_{b}")
                pacc = wpool.tile([1, SC], F32, tag="pacc", name=f"pacc_{b}")
                ctx_c = [wpool.tile([128, HT], F32, tag="ctxc", bufs=8, name=f"ctxc_{b}_{c}")
                         for c in range(SC)]
                return p32, pacc, ctx_c

            def kp_group(b, c, ot, eT):
                """one k_proj group: fp8 DoubleRow main + two residual passes
                (all at the same 4096x scale, accumulated in one psum), then
                tanh with the 1/4096 undo folded into the activation scale."""
                ps = kp_ps.tile([128, 512], F32, tag="kp", name=f"kp_ps{b}_{c}_{ot}")
                t8, t8r = k8_sb[(b, c)]
                ua8v = ua8_sb.rearrange("p (t k o) -> p t k o", t=HT, k=HT)
                ua8rv = ua8r_sb.rearrange("p (t k o) -> p t k o", t=HT, k=HT)
                k8v = t8.rearrange("p (k s) -> p k s", k=HT)
                k8rv = t8r.rearrange("p (k s) -> p k s", k=HT)
                passes = [(ua8v, k8v), (ua8v, k8rv), (ua8rv, k8v)]
                n = len(passes) * (HT // 2)
                i = 0
                for uav, kv in passes:
                    for j in range(HT // 2):
                        nc.tensor.matmul(
                            ps[:],
                            lhsT=uav[:, ot, 2 * j:2 * j + 2, :],
                            rhs=kv[:, 2 * j:2 * j + 2, :],
                            start=(i == 0), stop=(i == n - 1),
                            perf_mode=mybir.MatmulPerfMode.DoubleRow,
                        )
                        i += 1
                nc.scalar.activation(
                    eT[:, ot * 512:(ot + 1) * 512], ps[:],
                    AF.Tanh, scale=INV_SCALE,
                    bias=qp_sb[:, ot * BPC + b: ot * BPC + b + 1],
                )

            def scores_mms(b, c, eT):
                """scores = Va . e on DVE: chained (eT[ot] * va[ot]) + acc,
                then one GPSIMD all-reduce across the 128 partitions."""
                acc = None
                for ot in range(HT):
                    nacc = wpool.tile([128, 512], F32, tag="sacc", bufs=3,
                                      name=f"sacc_{b}_{c}_{ot}")
                    if acc is None:
                        nc.vector.tensor_scalar_mul(
                            nacc[:], eT[:, ot * 512:(ot + 1) * 512], va_sb[:, ot:ot + 1])
                    else:
                        nc.vector.scalar_tensor_tensor(
                            out=nacc[:], in0=eT[:, ot * 512:(ot + 1) * 512],
                            scalar=va_sb[:, ot:ot + 1], in1=acc[:],
                            op0=ALU.mult, op1=ALU.add)
                    acc = nacc
                sred = wpool.tile([128, 512], F32, tag="sred", bufs=2,
                                  name=f"sred_{b}_{c}")
                nc.gpsimd.partition_all_reduce(sred[:], acc[:], channels=128,
                                               reduce_op=bass_isa.ReduceOp.add)
                return sred

            def exp_block(b, c, sps):
                """p = exp(scores) chunk + running denominator + bf16 copy."""
                p32, pacc, ctx_c = state[b]
                nc.scalar.activation(
                    p32[:, c * 512:(c + 1) * 512], sps[0:1, :],
                    AF.Exp, accum_out=pacc[:, c:c + 1],
                )
                p16 = wpool.tile([1, 512], BF16, tag="p16", bufs=3, name=f"p16_{b}_{c}")
                nc.scalar.copy(p16[:], p32[:, c * 512:(c + 1) * 512])
                return p16

            def ctx_block(b, c, p16):
                """replicate p across partitions, then per-chunk context on DVE."""
                p32, pacc, ctx_c = state[b]
                rep_ps = misc_ps.tile([128, 512], F32, tag="mps", name=f"rep_ps{b}_{c}")
                nc.tensor.matmul(rep_ps[:], lhsT=ones_sb[:], rhs=p16[:])
                prep = wpool.tile([128, 512], BF16, tag="prep", bufs=3, name=f"prep_{b}_{c}")
                nc.vector.tensor_copy(prep[:], rep_ps[:])
                # ctx_c[c][:, ht] = sum_s keysT[ht][:, s] * p[s] over this chunk
                for ht in range(HT):
                    tout = wpool.tile([128, 512], BF16, tag="ttro", name=f"ttro_{b}_{c}_{ht}")
                    nc.vector.scalar_tensor_tensor(
                        out=tout[:],
                        in0=keys_sb[b][:, (c * HT + ht) * 512: (c * HT + ht) * 512 + 512],
                        scalar=1.0,
                        in1=prep[:],
                        op0=ALU.bypass, op1=ALU.mult,
                        accum_out=ctx_c[c][:, ht:ht + 1],
                    )

            def finalize(b):
                """normalize: weights out, context = (sum_c ctx_c) / sum(p)."""
                p32, pacc, ctx_c = state[b]
                l_t = wpool.tile([1, 1], F32, tag="l", name=f"l_{b}")
                nc.vector.tensor_reduce(l_t[:], pacc[:], axis=mybir.AxisListType.X, op=ALU.add)
                r_t = wpool.tile([1, 1], F32, tag="r", name=f"r_{b}")
                nc.vector.reciprocal(r_t[:], l_t[:])
                w_t = wpool.tile([1, S], F32, tag="w", name=f"w_{b}")
                nc.scalar.mul(w_t[:], p32[:], r_t[:])
                nc.sync.dma_start(out=wout_d[b], in_=w_t[:])
                # replicate r across partitions (fp32 K=1 matmul), scale context
                rep2 = misc_ps.tile([128, 512], F32, tag="mps", name=f"rep2_{b}")
                nc.tensor.matmul(rep2[:, 0:1], lhsT=ones32_sb[:], rhs=r_t[:])
                r128 = wpool.tile([128, 1], F32, tag="r128", name=f"r128_{b}")
                nc.vector.tensor_copy(r128[:], rep2[:, 0:1])
                s01 = wpool.tile([128, HT], F32, tag="s01", name=f"s01_{b}")
                nc.vector.tensor_tensor(out=s01[:], in0=ctx_c[0][:], in1=ctx_c[1][:], op=ALU.add)
                s23 = wpool.tile([128, HT], F32, tag="s23", name=f"s23_{b}")
                nc.vector.tensor_tensor(out=s23[:], in0=ctx_c[2][:], in1=ctx_c[3][:], op=ALU.add)
                csum = wpool.tile([128, HT], F32, tag="csum", name=f"csum_{b}")
                nc.vector.tensor_tensor(out=csum[:], in0=s01[:], in1=s23[:], op=ALU.add)
                ctx = wpool.tile([128, HT], F32, tag="ctx", name=f"ctx_{b}")
                nc.vector.tensor_scalar_mul(ctx[:], csum[:], r128[:])
                nc.sync.dma_start(out=ctxout_d[b], in_=ctx[:])

            # ---- main loop ----
            # The previous chunk's scores / exp / context work is interleaved
            # between this chunk's kp groups so that by the time PE reaches
            # each dependent matmul (scores, replicate) its ACT/DVE inputs are
            # already done — keeps the PE stream stall-free.
            pending = None
            for b in range(BPC):
                state[b] = batch_state(b)
                if b >= 1 and b + 1 < BPC:
                    load_keys(b + 1)
                for c in range(SC):
                    n = b * SC + c + 2
                    if n < BPC * SC:
                        load_k8(n // SC, n % SC)
                    eT = wpool.tile([128, HT * 512], BF16, tag="eT", bufs=3,
                                    name=f"eT_{b}_{c}")
                    for ot in range(HT):
                        if b == 0 and c == 0:
                            qp_group(ot)
                        kp_group(b, c, ot, eT)
                        if b == 0 and c == 0 and ot == HT - 1:
                            load_keys(1)
                        if pending is not None:
                            pb, pc, peT, psps, pp16 = pending
                            if ot == 0:
                                psps = scores_mms(pb, pc, peT)
                                pending = (pb, pc, peT, psps, pp16)
                            elif ot == 1:
                                pp16 = exp_block(pb, pc, psps)
                                pending = (pb, pc, peT, psps, pp16)
                            elif ot == 2:
                                ctx_block(pb, pc, pp16)
                            elif ot == 4 and pc == SC - 1:
                                finalize(pb)
                    pending = (b, c, eT, None, None)
            pb, pc, peT, _, _ = pending
            # final chunk: scores on PE (the DVE pipeline has no later kp work
            # to hide behind, and the PE is idle here anyway)
            fps = misc_ps.tile([128, 512], F32, tag="mps", name="final_s_ps")
            for ot in range(HT):
                nc.tensor.matmul(
                    fps[0:1, :],
                    lhsT=va16_sb[:, ot:ot + 1],
                    rhs=peT[:, ot * 512:(ot + 1) * 512],
                    start=(ot == 0), stop=(ot == HT - 1),
                )
            p16 = exp_block(pb, pc, fps)
            ctx_block(pb, pc, p16)
            finalize(pb)

    nc.compile()
    return nc


def _prep_inputs(query, keys, Wa_w, Wa_b, Ua_w, Ua_b, Va_w, Va_b):
    bf16 = ml_dtypes.bfloat16
    # Va_b shifts every score equally; softmax is shift-invariant and scores
    # themselves are not returned, so it never affects the output.
    del Va_b
    e4 = ml_dtypes.float8_e4m3fn
    uo = np.ascontiguousarray(
        (Ua_w.T * USCALE).reshape(HT, 128, HT, 128).transpose(2, 1, 0, 3)
    ).reshape(HT, 128, H).astype(np.float32)
    ua8 = uo.astype(e4)
    ua8r = (uo - ua8.astype(np.float32)).astype(e4)
    waT = np.ascontiguousarray(
        Wa_w.T.reshape(HT, 128, HT, 128).transpose(2, 1, 0, 3)).reshape(HT, 128, H).astype(bf16)
    vaT = Va_w[0].reshape(HT, 128).astype(np.float32)
    wbT = np.ascontiguousarray(Wa_b.reshape(HT, 128).T).astype(np.float32)
    ubT = np.ascontiguousarray(Ua_b.reshape(HT, 128).T).astype(np.float32)
    ones = np.ones((1, 128), dtype=bf16)
    ones32 = np.ones((1, 128), dtype=np.float32)

    in_maps = []
    for core in range(N_CORES):
        bs = slice(core * BPC, (core + 1) * BPC)
        kc = np.ascontiguousarray(
            keys[bs].reshape(BPC, SC, 512, HT, 128).transpose(0, 1, 4, 3, 2)
        ).reshape(BPC, SC, 128, HT * 512)
        kT = kc.astype(bf16)
        k16 = (kc * KSCALE).astype(np.float32)
        k8 = k16.astype(e4)
        k8r = (k16 - k8.astype(np.float32)).astype(e4)
        qT = np.ascontiguousarray(
            query[bs, 0, :].T).reshape(HT, 128, BPC).astype(bf16)
        in_maps.append({
            "keysT": kT, "keys8": k8, "keys8r": k8r, "qT": qT, "ua8": ua8,
            "ua8r": ua8r, "waT": waT, "vaT": vaT,
            "wbT": wbT, "ubT": ubT, "ones": ones, "ones32": ones32,
        })
    return in_maps


def kernel(query, keys, Wa_w, Wa_b, Ua_w, Ua_b, Va_w, Va_b):
    if "nc" not in _cache:
        _cache["nc"] = build_module()
    nc = _cache["nc"]

    in_maps = _prep_inputs(query, keys, Wa_w, Wa_b, Ua_w, Ua_b, Va_w, Va_b)
    res = bass_utils.run_bass_kernel_spmd(nc, in_maps, core_ids=list(range(N_CORES)))

    context = np.empty((B, 1, H), dtype=np.float32)
    weights = np.empty((B, 1, S), dtype=np.float32)
    for core in range(N_CORES):
        out = res.results[core]
        weights[core * BPC:(core + 1) * BPC, 0, :] = out["wout"]
        # ctxout[b, p, t] holds context[h = t*128 + p]
        context[core * BPC:(core + 1) * BPC, 0, :] = (
            out["ctxout"].transpose(0, 2, 1).reshape(BPC, H))
    return (context, weights)


# revision 25
# speedup vs baseline: 1.1959x; 1.0202x over previous
"""Bahdanau attention on 8 Trainium2 NeuronCores.

Reference computation (B=32, S=2048, H=1024, fp32):
    q_proj = query @ Wa_w.T + Wa_b            # [B,1,H]
    k_proj = keys @ Ua_w.T + Ua_b             # [B,S,H]
    e      = tanh(q_proj + k_proj)            # [B,S,H]
    scores = e @ Va_w.T + Va_b                # [B,S,1] -> [B,1,S]
    weights = softmax(scores, axis=-1)
    context = weights @ keys                  # [B,1,H]
    returns (context, weights)

Sharding: data-parallel over batch, 4 batches per core; the small
Wa/Ua/Va weights are replicated.

Per-core dataflow (all matmuls contract over the partition dim):
  - keys are uploaded pre-transposed per batch as keysT[h, s] (bf16), so the
    big k_proj matmul runs with Ua_w.T chunks stationary and keysT streaming:
    out[o_tile(128p), s_chunk(512)] accumulates over 8 h-tiles in PSUM.
  - tanh(k_proj + q_proj[o] + Wa_b[o] + Ua_b[o]) is one ScalarE activation per
    tile: the whole per-o bias vector is the per-partition activation bias.
  - scores = Va . e is 8 accumulating matmuls with Va chunk stationary (M=1).
  - softmax skips the max-subtraction (scores are bounded by sum|Va| ~ 16, so
    exp cannot overflow in fp32; softmax is shift-invariant so the result is
    identical to the reference up to rounding).  exp runs on ScalarE with
    accum_out giving the denominator chunks for free.
  - context accumulates per chunk with UNNORMALIZED p = exp(scores): p is
    replicated across partitions by a ones-vector matmul, then
    sum_s p[s] * keysT[h, s] runs on VectorE as fused scalar_tensor_tensor
    ops against the SBUF-resident keysT.  The 1/sum(p) normalization is
    applied once at the end (context scale + weights output), so the context
    reduction overlaps the remaining k_proj matmuls instead of serializing
    after the whole batch.
"""

import numpy as np
import ml_dtypes

import concourse.bass as bass
import concourse.mybir as mybir
import concourse.tile as tile
import concourse.bass_isa as bass_isa
from concourse import bacc
from concourse import bass_utils

BF16 = mybir.dt.bfloat16
F32 = mybir.dt.float32
FP8 = mybir.dt.float8e4
KSCALE = 16.0
USCALE = 256.0
INV_SCALE = 1.0 / (KSCALE * USCALE)
AF = mybir.ActivationFunctionType
ALU = mybir.AluOpType

N_CORES = 8
B, S, H = 32, 2048, 1024
BPC = B // N_CORES          # batches per core
HT = H // 128               # h (and o) tiles of 128
SC = S // 512               # s chunks of 512

_cache = {}


def build_module():
    nc = bacc.Bacc("TRN2", target_bir_lowering=False, debug=False,
                   enable_asserts=False, num_devices=N_CORES)

    # Per-core inputs
    keysT_d = nc.dram_tensor("keysT", [BPC, SC, 128, HT * 512], BF16, kind="ExternalInput").ap()
    qT_d = nc.dram_tensor("qT", [HT, 128, BPC], BF16, kind="ExternalInput").ap()
    keys8_d = nc.dram_tensor("keys8", [BPC, SC, 128, HT * 512], FP8, kind="ExternalInput").ap()
    keys8r_d = nc.dram_tensor("keys8r", [BPC, SC, 128, HT * 512], FP8, kind="ExternalInput").ap()
    # Replicated weights ([ot, p, ht*128+o_in] o-major pieces)
    ua8_d = nc.dram_tensor("ua8", [HT, 128, H], FP8, kind="ExternalInput").ap()
    ua8r_d = nc.dram_tensor("ua8r", [HT, 128, H], FP8, kind="ExternalInput").ap()
    waT_d = nc.dram_tensor("waT", [HT, 128, H], BF16, kind="ExternalInput").ap()
    vaT_d = nc.dram_tensor("vaT", [HT, 128], F32, kind="ExternalInput").ap()
    wbT_d = nc.dram_tensor("wbT", [128, HT], F32, kind="ExternalInput").ap()
    ubT_d = nc.dram_tensor("ubT", [128, HT], F32, kind="ExternalInput").ap()
    ones_d = nc.dram_tensor("ones", [1, 128], BF16, kind="ExternalInput").ap()
    ones32_d = nc.dram_tensor("ones32", [1, 128], F32, kind="ExternalInput").ap()
    # Outputs
    wout_d = nc.dram_tensor("wout", [BPC, S], F32, kind="ExternalOutput").ap()
    ctxout_d = nc.dram_tensor("ctxout", [BPC, 128, HT], F32, kind="ExternalOutput").ap()

    with tile.TileContext(nc) as tc:
        with (
            tc.tile_pool(name="const", bufs=1) as cpool,
            tc.tile_pool(name="keys", bufs=2) as kpool,
            tc.tile_pool(name="work", bufs=2) as wpool,
            tc.tile_pool(name="kp_ps", bufs=6, space="PSUM") as kp_ps,
            tc.tile_pool(name="misc_ps", bufs=2, space="PSUM") as misc_ps,
        ):
            # ---- constants into SBUF ----
            # DMA order is tuned so PE never waits: qp inputs (q, wa piece 0)
            # first, then ua piece 0 + keys0 chunk 0 (first kp group), then the
            # remaining wa/ua pieces interleaved, with keys0 chunks spread in
            # between at the rate the kp groups consume them.
            q_sb = cpool.tile([128, HT * BPC], BF16, tag="q")
            nc.sync.dma_start(out=q_sb[:], in_=qT_d.rearrange("t p b -> p t b"))
            wa_sb = cpool.tile([128, HT * H], BF16, tag="wa")
            nc.sync.dma_start(out=wa_sb[:, 0:H], in_=waT_d[0])
            ua8_sb = cpool.tile([128, HT * H], FP8, tag="ua8")
            nc.sync.dma_start(out=ua8_sb[:, 0:H], in_=ua8_d[0])
            ua8r_sb = cpool.tile([128, HT * H], FP8, tag="ua8r")
            nc.sync.dma_start(out=ua8r_sb[:, 0:H], in_=ua8r_d[0])

            keys_sb = {}
            k8_sb = {}

            def load_k8(b, c):
                t8 = wpool.tile([128, HT * 512], FP8, tag="k8", bufs=3,
                                name=f"k8_{b}_{c}")
                nc.sync.dma_start(out=t8[:], in_=keys8_d[b, c])
                t8r = wpool.tile([128, HT * 512], FP8, tag="k8r", bufs=3,
                                 name=f"k8r_{b}_{c}")
                nc.sync.dma_start(out=t8r[:], in_=keys8r_d[b, c])
                k8_sb[(b, c)] = (t8, t8r)

            load_k8(0, 0)

            va_sb = cpool.tile([128, HT], F32, tag="va")
            nc.sync.dma_start(out=va_sb[:], in_=vaT_d.rearrange("t p -> p t"))
            va16_sb = cpool.tile([128, HT], BF16, tag="va16")
            nc.vector.tensor_copy(va16_sb[:], va_sb[:])
            ones_sb = cpool.tile([1, 128], BF16, tag="ones")
            nc.sync.dma_start(out=ones_sb[:], in_=ones_d)
            ones32_sb = cpool.tile([1, 128], F32, tag="ones32")
            nc.sync.dma_start(out=ones32_sb[:], in_=ones32_d)
            wb_sb = cpool.tile([128, HT], F32, tag="wb")
            nc.sync.dma_start(out=wb_sb[:], in_=wbT_d)
            ub_sb = cpool.tile([128, HT], F32, tag="ub")
            nc.sync.dma_start(out=ub_sb[:], in_=ubT_d)
            bias_sb = cpool.tile([128, HT], F32, tag="bias")
            nc.vector.tensor_tensor(out=bias_sb[:], in0=wb_sb[:], in1=ub_sb[:], op=ALU.add)

            def load_keys(b):
                # chunk-major layout: [:, c*HT*512 + ht*512 + s]
                t = kpool.tile([128, SC * HT * 512], BF16, tag="keys", name=f"keys{b}")
                for c in range(SC):
                    nc.sync.dma_start(
                        out=t[:, c * HT * 512:(c + 1) * HT * 512], in_=keysT_d[b, c])
                keys_sb[b] = t

            for ot in range(1, HT):
                nc.sync.dma_start(out=wa_sb[:, ot * H:(ot + 1) * H], in_=waT_d[ot])
                nc.sync.dma_start(out=ua8_sb[:, ot * H:(ot + 1) * H], in_=ua8_d[ot])
                nc.sync.dma_start(out=ua8r_sb[:, ot * H:(ot + 1) * H], in_=ua8r_d[ot])
                if ot == 4:
                    load_k8(0, 1)
            load_keys(0)

            # q_proj group: qp_sb[:, ot*BPC + b] = (Wa_w @ q + Wa_b + Ua_b)[o]
            qp_sb = cpool.tile([128, HT * BPC], F32, tag="qp")

            def qp_group(ot):
                ps = misc_ps.tile([128, 512], F32, tag="mps", name=f"qp_ps{ot}")
                for ht in range(HT):
                    nc.tensor.matmul(
                        ps[:, :BPC],
                        lhsT=wa_sb[:, ot * H + ht * 128: ot * H + ht * 128 + 128],
                        rhs=q_sb[:, ht * BPC:(ht + 1) * BPC],
                        start=(ht == 0), stop=(ht == HT - 1),
                    )
                nc.scalar.activation(
                    qp_sb[:, ot * BPC:(ot + 1) * BPC], ps[:, :BPC],
                    AF.Identity, bias=bias_sb[:, ot:ot + 1],
                )

            # ---- per-batch state ----
            state = {}

            def batch_state(b):
                p32 = wpool.tile([1, S], F32, tag="p32", name=f"p32_{b}")
                pacc = wpool.tile([1, SC], F32, tag="pacc", name=f"pacc_{b}")
                ctx_c = [wpool.tile([128, HT], F32, tag="ctxc", bufs=8, name=f"ctxc_{b}_{c}")
                         for c in range(SC)]
                return p32, pacc, ctx_c

            def kp_group(b, c, ot, eT):
                """one k_proj group: fp8 DoubleRow main + two residual passes
                (all at the same 4096x scale, accumulated in one psum), then
                tanh with the 1/4096 undo folded into the activation scale."""
                ps = kp_ps.tile([128, 512], F32, tag="kp", name=f"kp_ps{b}_{c}_{ot}")
                t8, t8r = k8_sb[(b, c)]
                ua8v = ua8_sb.rearrange("p (t k o) -> p t k o", t=HT, k=HT)
                ua8rv = ua8r_sb.rearrange("p (t k o) -> p t k o", t=HT, k=HT)
                k8v = t8.rearrange("p (k s) -> p k s", k=HT)
                k8rv = t8r.rearrange("p (k s) -> p k s", k=HT)
                passes = [(ua8v, k8v), (ua8v, k8rv), (ua8rv, k8v)]
                n = len(passes) * (HT // 2)
                i = 0
                for uav, kv in passes:
                    for j in range(HT // 2):
                        nc.tensor.matmul(
                            ps[:],
                            lhsT=uav[:, ot, 2 * j:2 * j + 2, :],
                            rhs=kv[:, 2 * j:2 * j + 2, :],
                            start=(i == 0), stop=(i == n - 1),
                            perf_mode=mybir.MatmulPerfMode.DoubleRow,
                        )
                        i += 1
                nc.scalar.activation(
                    eT[:, ot * 512:(ot + 1) * 512], ps[:],
                    AF.Tanh, scale=INV_SCALE,
                    bias=qp_sb[:, ot * BPC + b: ot * BPC + b + 1],
                )

            def scores_mms(b, c, eT):
                """scores = Va . e on DVE: chained (eT[ot] * va[ot]) + acc,
                then one GPSIMD all-reduce across the 128 partitions."""
                acc = None
                for ot in range(HT):
                    nacc = wpool.tile([128, 512], F32, tag="sacc", bufs=3,
                                      name=f"sacc_{b}_{c}_{ot}")
                    if acc is None:
                        nc.vector.tensor_scalar_mul(
                            nacc[:], eT[:, ot * 512:(ot + 1) * 512], va_sb[:, ot:ot + 1])
                    else:
                        nc.vector.scalar_tensor_tensor(
                            out=nacc[:], in0=eT[:, ot * 512:(ot + 1) * 512],
                            scalar=va_sb[:, ot:ot + 1], in1=acc[:],
                            op0=ALU.mult, op1=ALU.add)
                    acc = nacc
                sred = wpool.tile([128, 512], F32, tag="sred", bufs=2,
                                  name=f"sred_{b}_{c}")
                nc.gpsimd.partition_all_reduce(sred[:], acc[:], channels=128,
                                               reduce_op=bass_isa.ReduceOp.add)
                return sred

            def exp_block(b, c, sps):
                """p = exp(scores) chunk + running denominator + bf16 copy."""
                p32, pacc, ctx_c = state[b]
                nc.scalar.activation(
                    p32[:, c * 512:(c + 1) * 512], sps[0:1, :],
                    AF.Exp, accum_out=pacc[:, c:c + 1],
                )
                p16 = wpool.tile([1, 512], BF16, tag="p16", bufs=3, name=f"p16_{b}_{c}")
                nc.scalar.copy(p16[:], p32[:, c * 512:(c + 1) * 512])
                return p16

            def ctx_block(b, c, p16):
                """replicate p across partitions, then per-chunk context on DVE."""
                p32, pacc, ctx_c = state[b]
                rep_ps = misc_ps.tile([128, 512], F32, tag="mps", name=f"rep_ps{b}_{c}")
                nc.tensor.matmul(rep_ps[:], lhsT=ones_sb[:], rhs=p16[:])
                prep = wpool.tile([128, 512], BF16, tag="prep", bufs=3, name=f"prep_{b}_{c}")
                nc.vector.tensor_copy(prep[:], rep_ps[:])
                # ctx_c[c][:, ht] = sum_s keysT[ht][:, s] * p[s] over this chunk
                for ht in range(HT):
                    tout = wpool.tile([128, 512], BF16, tag="ttro", name=f"ttro_{b}_{c}_{ht}")
                    nc.vector.scalar_tensor_tensor(
                        out=tout[:],
                        in0=keys_sb[b][:, (c * HT + ht) * 512: (c * HT + ht) * 512 + 512],
                        scalar=1.0,
                        in1=prep[:],
                        op0=ALU.bypass, op1=ALU.mult,
                        accum_out=ctx_c[c][:, ht:ht + 1],
                    )

            def finalize(b):
                """normalize: weights out, context = (sum_c ctx_c) / sum(p)."""
                p32, pacc, ctx_c = state[b]
                l_t = wpool.tile([1, 1], F32, tag="l", name=f"l_{b}")
                nc.vector.tensor_reduce(l_t[:], pacc[:], axis=mybir.AxisListType.X, op=ALU.add)
                r_t = wpool.tile([1, 1], F32, tag="r", name=f"r_{b}")
                nc.vector.reciprocal(r_t[:], l_t[:])
                w_t = wpool.tile([1, S], F32, tag="w", name=f"w_{b}")
                nc.scalar.mul(w_t[:], p32[:], r_t[:])
                nc.sync.dma_start(out=wout_d[b], in_=w_t[:])
                # replicate r across partitions (fp32 K=1 matmul), scale context
                rep2 = misc_ps.tile([128, 512], F32, tag="mps", name=f"rep2_{b}")
                nc.tensor.matmul(rep2[:, 0:1], lhsT=ones32_sb[:], rhs=r_t[:])
                r128 = wpool.tile([128, 1], F32, tag="r128", name=f"r128_{b}")
                nc.vector.tensor_copy(r128[:], rep2[:, 0:1])
                s01 = wpool.tile([128, HT], F32, tag="s01", name=f"s01_{b}")
                nc.vector.tensor_tensor(out=s01[:], in0=ctx_c[0][:], in1=ctx_c[1][:], op=ALU.add)
                s23 = wpool.tile([128, HT], F32, tag="s23", name=f"s23_{b}")
                nc.vector.tensor_tensor(out=s23[:], in0=ctx_c[2][:], in1=ctx_c[3][:], op=ALU.add)
                csum = wpool.tile([128, HT], F32, tag="csum", name=f"csum_{b}")
                nc.vector.tensor_tensor(out=csum[:], in0=s01[:], in1=s23[:], op=ALU.add)
                ctx = wpool.tile([128, HT], F32, tag="ctx", name=f"ctx_{b}")
                nc.vector.tensor_scalar_mul(ctx[:], csum[:], r128[:])
                nc.sync.dma_start(out=ctxout_d[b], in_=ctx[:])

            # ---- main loop ----
            # The previous chunk's scores / exp / context work is interleaved
            # between this chunk's kp groups so that by the time PE reaches
            # each dependent matmul (scores, replicate) its ACT/DVE inputs are
            # already done — keeps the PE stream stall-free.
            pending = None
            for b in range(BPC):
                state[b] = batch_state(b)
                for c in range(SC):
                    n = b * SC + c + 2
                    if n < BPC * SC:
                        load_k8(n // SC, n % SC)
                    # keysT only feeds the ctx reduction (trails kp by a
                    # chunk), so its chunks trickle in behind the k8 loads:
                    # (b+1, 0..2) during (b, 1..3); (b, 3) during (b, 0).
                    if b + 1 < BPC and c >= 1:
                        if c == 1:
                            keys_sb[b + 1] = kpool.tile(
                                [128, SC * HT * 512], BF16, tag="keys",
                                name=f"keys{b + 1}")
                        nc.sync.dma_start(
                            out=keys_sb[b + 1][:, (c - 1) * HT * 512: c * HT * 512],
                            in_=keysT_d[b + 1, c - 1])
                    if b >= 1 and c == 0:
                        nc.sync.dma_start(
                            out=keys_sb[b][:, 3 * HT * 512: 4 * HT * 512],
                            in_=keysT_d[b, 3])
                    eT = wpool.tile([128, HT * 512], BF16, tag="eT", bufs=3,
                                    name=f"eT_{b}_{c}")
                    for ot in range(HT):
                        if b == 0 and c == 0:
                            qp_group(ot)
                        kp_group(b, c, ot, eT)
                        if pending is not None:
                            pb, pc, peT, psps, pp16 = pending
                            if ot == 0:
                                psps = scores_mms(pb, pc, peT)
                                pending = (pb, pc, peT, psps, pp16)
                            elif ot == 1:
                                pp16 = exp_block(pb, pc, psps)
                                pending = (pb, pc, peT, psps, pp16)
                            elif ot == 2:
                                ctx_block(pb, pc, pp16)
                            elif ot == 4 and pc == SC - 1:
                                finalize(pb)
                    pending = (b, c, eT, None, None)
            pb, pc, peT, _, _ = pending
            # final chunk: scores on PE (the DVE pipeline has no later kp work
            # to hide behind, and the PE is idle here anyway)
            fps = misc_ps.tile([128, 512], F32, tag="mps", name="final_s_ps")
            for ot in range(HT):
                nc.tensor.matmul(
                    fps[0:1, :],
                    lhsT=va16_sb[:, ot:ot + 1],
                    rhs=peT[:, ot * 512:(ot + 1) * 512],
                    start=(ot == 0), stop=(ot == HT - 1),
                )
            p16 = exp_block(pb, pc, fps)
            ctx_block(pb, pc, p16)
            finalize(pb)

    nc.compile()
    return nc


def _prep_inputs(query, keys, Wa_w, Wa_b, Ua_w, Ua_b, Va_w, Va_b):
    bf16 = ml_dtypes.bfloat16
    # Va_b shifts every score equally; softmax is shift-invariant and scores
    # themselves are not returned, so it never affects the output.
    del Va_b
    e4 = ml_dtypes.float8_e4m3fn
    uo = np.ascontiguousarray(
        (Ua_w.T * USCALE).reshape(HT, 128, HT, 128).transpose(2, 1, 0, 3)
    ).reshape(HT, 128, H).astype(np.float32)
    ua8 = uo.astype(e4)
    ua8r = (uo - ua8.astype(np.float32)).astype(e4)
    waT = np.ascontiguousarray(
        Wa_w.T.reshape(HT, 128, HT, 128).transpose(2, 1, 0, 3)).reshape(HT, 128, H).astype(bf16)
    vaT = Va_w[0].reshape(HT, 128).astype(np.float32)
    wbT = np.ascontiguousarray(Wa_b.reshape(HT, 128).T).astype(np.float32)
    ubT = np.ascontiguousarray(Ua_b.reshape(HT, 128).T).astype(np.float32)
    ones = np.ones((1, 128), dtype=bf16)
    ones32 = np.ones((1, 128), dtype=np.float32)

    in_maps = []
    for core in range(N_CORES):
        bs = slice(core * BPC, (core + 1) * BPC)
        kc = np.ascontiguousarray(
            keys[bs].reshape(BPC, SC, 512, HT, 128).transpose(0, 1, 4, 3, 2)
        ).reshape(BPC, SC, 128, HT * 512)
        kT = kc.astype(bf16)
        k16 = (kc * KSCALE).astype(np.float32)
        k8 = k16.astype(e4)
        k8r = (k16 - k8.astype(np.float32)).astype(e4)
        qT = np.ascontiguousarray(
            query[bs, 0, :].T).reshape(HT, 128, BPC).astype(bf16)
        in_maps.append({
            "keysT": kT, "keys8": k8, "keys8r": k8r, "qT": qT, "ua8": ua8,
            "ua8r": ua8r, "waT": waT, "vaT": vaT,
            "wbT": wbT, "ubT": ubT, "ones": ones, "ones32": ones32,
        })
    return in_maps


def kernel(query, keys, Wa_w, Wa_b, Ua_w, Ua_b, Va_w, Va_b):
    if "nc" not in _cache:
        _cache["nc"] = build_module()
    nc = _cache["nc"]

    in_maps = _prep_inputs(query, keys, Wa_w, Wa_b, Ua_w, Ua_b, Va_w, Va_b)
    res = bass_utils.run_bass_kernel_spmd(nc, in_maps, core_ids=list(range(N_CORES)))

    context = np.empty((B, 1, H), dtype=np.float32)
    weights = np.empty((B, 1, S), dtype=np.float32)
    for core in range(N_CORES):
        out = res.results[core]
        weights[core * BPC:(core + 1) * BPC, 0, :] = out["wout"]
        # ctxout[b, p, t] holds context[h = t*128 + p]
        context[core * BPC:(core + 1) * BPC, 0, :] = (
            out["ctxout"].transpose(0, 2, 1).reshape(BPC, H))
    return (context, weights)


# revision 26
# speedup vs baseline: 1.1971x; 1.0010x over previous
"""Bahdanau attention on 8 Trainium2 NeuronCores.

Reference computation (B=32, S=2048, H=1024, fp32):
    q_proj = query @ Wa_w.T + Wa_b            # [B,1,H]
    k_proj = keys @ Ua_w.T + Ua_b             # [B,S,H]
    e      = tanh(q_proj + k_proj)            # [B,S,H]
    scores = e @ Va_w.T + Va_b                # [B,S,1] -> [B,1,S]
    weights = softmax(scores, axis=-1)
    context = weights @ keys                  # [B,1,H]
    returns (context, weights)

Sharding: data-parallel over batch, 4 batches per core; the small
Wa/Ua/Va weights are replicated.

Per-core dataflow (all matmuls contract over the partition dim):
  - keys are uploaded pre-transposed per batch as keysT[h, s] (bf16), so the
    big k_proj matmul runs with Ua_w.T chunks stationary and keysT streaming:
    out[o_tile(128p), s_chunk(512)] accumulates over 8 h-tiles in PSUM.
  - tanh(k_proj + q_proj[o] + Wa_b[o] + Ua_b[o]) is one ScalarE activation per
    tile: the whole per-o bias vector is the per-partition activation bias.
  - scores = Va . e is 8 accumulating matmuls with Va chunk stationary (M=1).
  - softmax skips the max-subtraction (scores are bounded by sum|Va| ~ 16, so
    exp cannot overflow in fp32; softmax is shift-invariant so the result is
    identical to the reference up to rounding).  exp runs on ScalarE with
    accum_out giving the denominator chunks for free.
  - context accumulates per chunk with UNNORMALIZED p = exp(scores): p is
    replicated across partitions by a ones-vector matmul, then
    sum_s p[s] * keysT[h, s] runs on VectorE as fused scalar_tensor_tensor
    ops against the SBUF-resident keysT.  The 1/sum(p) normalization is
    applied once at the end (context scale + weights output), so the context
    reduction overlaps the remaining k_proj matmuls instead of serializing
    after the whole batch.
"""

import numpy as np
import ml_dtypes

import concourse.bass as bass
import concourse.mybir as mybir
import concourse.tile as tile
import concourse.bass_isa as bass_isa
from concourse import bacc
from concourse import bass_utils

BF16 = mybir.dt.bfloat16
F32 = mybir.dt.float32
FP8 = mybir.dt.float8e4
KSCALE = 16.0
USCALE = 256.0
INV_SCALE = 1.0 / (KSCALE * USCALE)
AF = mybir.ActivationFunctionType
ALU = mybir.AluOpType

N_CORES = 8
B, S, H = 32, 2048, 1024
BPC = B // N_CORES          # batches per core
HT = H // 128               # h (and o) tiles of 128
SC = S // 512               # s chunks of 512

_cache = {}


def build_module():
    nc = bacc.Bacc("TRN2", target_bir_lowering=False, debug=False,
                   enable_asserts=False, num_devices=N_CORES)

    # Per-core inputs
    keysT_d = nc.dram_tensor("keysT", [BPC, SC, 128, HT * 512], BF16, kind="ExternalInput").ap()
    qT_d = nc.dram_tensor("qT", [HT, 128, BPC], BF16, kind="ExternalInput").ap()
    keys8_d = nc.dram_tensor("keys8", [BPC, SC, 128, HT * 512], FP8, kind="ExternalInput").ap()
    keys8r_d = nc.dram_tensor("keys8r", [BPC, SC, 128, HT * 512], FP8, kind="ExternalInput").ap()
    # Replicated weights ([ot, p, ht*128+o_in] o-major pieces)
    ua8_d = nc.dram_tensor("ua8", [HT, 128, H], FP8, kind="ExternalInput").ap()
    ua8r_d = nc.dram_tensor("ua8r", [HT, 128, H], FP8, kind="ExternalInput").ap()
    waT_d = nc.dram_tensor("waT", [HT, 128, H], BF16, kind="ExternalInput").ap()
    vaT_d = nc.dram_tensor("vaT", [HT, 128], F32, kind="ExternalInput").ap()
    wbT_d = nc.dram_tensor("wbT", [128, HT], F32, kind="ExternalInput").ap()
    ubT_d = nc.dram_tensor("ubT", [128, HT], F32, kind="ExternalInput").ap()
    ones_d = nc.dram_tensor("ones", [1, 128], BF16, kind="ExternalInput").ap()
    ones32_d = nc.dram_tensor("ones32", [1, 128], F32, kind="ExternalInput").ap()
    # Outputs
    wout_d = nc.dram_tensor("wout", [BPC, S], F32, kind="ExternalOutput").ap()
    ctxout_d = nc.dram_tensor("ctxout", [BPC, 128, HT], F32, kind="ExternalOutput").ap()

    with tile.TileContext(nc) as tc:
        with (
            tc.tile_pool(name="const", bufs=1) as cpool,
            tc.tile_pool(name="keys", bufs=2) as kpool,
            tc.tile_pool(name="work", bufs=2) as wpool,
            tc.tile_pool(name="kp_ps", bufs=6, space="PSUM") as kp_ps,
            tc.tile_pool(name="misc_ps", bufs=2, space="PSUM") as misc_ps,
        ):
            # ---- constants into SBUF ----
            # DMA order is tuned so PE never waits: qp inputs (q, wa piece 0)
            # first, then ua piece 0 + keys0 chunk 0 (first kp group), then the
            # remaining wa/ua pieces interleaved, with keys0 chunks spread in
            # between at the rate the kp groups consume them.
            q_sb = cpool.tile([128, HT * BPC], BF16, tag="q")
            nc.sync.dma_start(out=q_sb[:], in_=qT_d.rearrange("t p b -> p t b"))
            wa_sb = cpool.tile([128, HT * H], BF16, tag="wa")
            nc.sync.dma_start(out=wa_sb[:, 0:H], in_=waT_d[0])
            ua8_sb = cpool.tile([128, HT * H], FP8, tag="ua8")
            nc.sync.dma_start(out=ua8_sb[:, 0:H], in_=ua8_d[0])
            ua8r_sb = cpool.tile([128, HT * H], FP8, tag="ua8r")
            nc.sync.dma_start(out=ua8r_sb[:, 0:H], in_=ua8r_d[0])

            keys_sb = {}
            k8_sb = {}

            def load_k8(b, c):
                t8 = wpool.tile([128, HT * 512], FP8, tag="k8", bufs=3,
                                name=f"k8_{b}_{c}")
                nc.sync.dma_start(out=t8[:], in_=keys8_d[b, c])
                t8r = wpool.tile([128, HT * 512], FP8, tag="k8r", bufs=3,
                                 name=f"k8r_{b}_{c}")
                nc.sync.dma_start(out=t8r[:], in_=keys8r_d[b, c])
                k8_sb[(b, c)] = (t8, t8r)

            load_k8(0, 0)

            va_sb = cpool.tile([128, HT], F32, tag="va")
            nc.sync.dma_start(out=va_sb[:], in_=vaT_d.rearrange("t p -> p t"))
            va16_sb = cpool.tile([128, HT], BF16, tag="va16")
            nc.vector.tensor_copy(va16_sb[:], va_sb[:])
            ones_sb = cpool.tile([1, 128], BF16, tag="ones")
            nc.sync.dma_start(out=ones_sb[:], in_=ones_d)
            ones32_sb = cpool.tile([1, 128], F32, tag="ones32")
            nc.sync.dma_start(out=ones32_sb[:], in_=ones32_d)
            wb_sb = cpool.tile([128, HT], F32, tag="wb")
            nc.sync.dma_start(out=wb_sb[:], in_=wbT_d)
            ub_sb = cpool.tile([128, HT], F32, tag="ub")
            nc.sync.dma_start(out=ub_sb[:], in_=ubT_d)
            bias_sb = cpool.tile([128, HT], F32, tag="bias")
            nc.vector.tensor_tensor(out=bias_sb[:], in0=wb_sb[:], in1=ub_sb[:], op=ALU.add)

            def load_keys(b):
                # chunk-major layout: [:, c*HT*512 + ht*512 + s]
                t = kpool.tile([128, SC * HT * 512], BF16, tag="keys", name=f"keys{b}")
                for c in range(SC):
                    nc.sync.dma_start(
                        out=t[:, c * HT * 512:(c + 1) * HT * 512], in_=keysT_d[b, c])
                keys_sb[b] = t

            for ot in range(1, HT):
                nc.sync.dma_start(out=wa_sb[:, ot * H:(ot + 1) * H], in_=waT_d[ot])
                nc.sync.dma_start(out=ua8_sb[:, ot * H:(ot + 1) * H], in_=ua8_d[ot])
                nc.sync.dma_start(out=ua8r_sb[:, ot * H:(ot + 1) * H], in_=ua8r_d[ot])
                if ot == 4:
                    load_k8(0, 1)
            load_keys(0)

            # q_proj group: qp_sb[:, ot*BPC + b] = (Wa_w @ q + Wa_b + Ua_b)[o]
            qp_sb = cpool.tile([128, HT * BPC], F32, tag="qp")

            def qp_group(ot):
                ps = misc_ps.tile([128, 512], F32, tag="mps", name=f"qp_ps{ot}")
                for ht in range(HT):
                    nc.tensor.matmul(
                        ps[:, :BPC],
                        lhsT=wa_sb[:, ot * H + ht * 128: ot * H + ht * 128 + 128],
                        rhs=q_sb[:, ht * BPC:(ht + 1) * BPC],
                        start=(ht == 0), stop=(ht == HT - 1),
                    )
                nc.scalar.activation(
                    qp_sb[:, ot * BPC:(ot + 1) * BPC], ps[:, :BPC],
                    AF.Identity, bias=bias_sb[:, ot:ot + 1],
                )

            # ---- per-batch state ----
            state = {}

            def batch_state(b):
                p32 = wpool.tile([1, S], F32, tag="p32", name=f"p32_{b}")
                pacc = wpool.tile([1, SC], F32, tag="pacc", name=f"pacc_{b}")
                ctx_c = [wpool.tile([128, HT], F32, tag="ctxc", bufs=8, name=f"ctxc_{b}_{c}")
                         for c in range(SC)]
                return p32, pacc, ctx_c

            def kp_group(b, c, ot, eT):
                """one k_proj group: fp8 DoubleRow main + two residual passes
                (all at the same 4096x scale, accumulated in one psum), then
                tanh with the 1/4096 undo folded into the activation scale."""
                ps = kp_ps.tile([128, 512], F32, tag="kp", name=f"kp_ps{b}_{c}_{ot}")
                t8, t8r = k8_sb[(b, c)]
                ua8v = ua8_sb.rearrange("p (t k o) -> p t k o", t=HT, k=HT)
                ua8rv = ua8r_sb.rearrange("p (t k o) -> p t k o", t=HT, k=HT)
                k8v = t8.rearrange("p (k s) -> p k s", k=HT)
                k8rv = t8r.rearrange("p (k s) -> p k s", k=HT)
                passes = [(ua8v, k8v), (ua8v, k8rv), (ua8rv, k8v)]
                n = len(passes) * (HT // 2)
                i = 0
                for uav, kv in passes:
                    for j in range(HT // 2):
                        nc.tensor.matmul(
                            ps[:],
                            lhsT=uav[:, ot, 2 * j:2 * j + 2, :],
                            rhs=kv[:, 2 * j:2 * j + 2, :],
                            start=(i == 0), stop=(i == n - 1),
                            perf_mode=mybir.MatmulPerfMode.DoubleRow,
                        )
                        i += 1
                nc.scalar.activation(
                    eT[:, ot * 512:(ot + 1) * 512], ps[:],
                    AF.Tanh, scale=INV_SCALE,
                    bias=qp_sb[:, ot * BPC + b: ot * BPC + b + 1],
                )

            def scores_mms(b, c, eT):
                """scores = Va . e on DVE: chained (eT[ot] * va[ot]) + acc,
                then one GPSIMD all-reduce across the 128 partitions."""
                acc = None
                for ot in range(HT):
                    nacc = wpool.tile([128, 512], F32, tag="sacc", bufs=3,
                                      name=f"sacc_{b}_{c}_{ot}")
                    if acc is None:
                        nc.vector.tensor_scalar_mul(
                            nacc[:], eT[:, ot * 512:(ot + 1) * 512], va_sb[:, ot:ot + 1])
                    else:
                        nc.vector.scalar_tensor_tensor(
                            out=nacc[:], in0=eT[:, ot * 512:(ot + 1) * 512],
                            scalar=va_sb[:, ot:ot + 1], in1=acc[:],
                            op0=ALU.mult, op1=ALU.add)
                    acc = nacc
                sred = wpool.tile([128, 512], F32, tag="sred", bufs=2,
                                  name=f"sred_{b}_{c}")
                nc.gpsimd.partition_all_reduce(sred[:], acc[:], channels=128,
                                               reduce_op=bass_isa.ReduceOp.add)
                return sred

            def exp_block(b, c, sps):
                """p = exp(scores) chunk + running denominator + bf16 copy."""
                p32, pacc, ctx_c = state[b]
                nc.scalar.activation(
                    p32[:, c * 512:(c + 1) * 512], sps[0:1, :],
                    AF.Exp, accum_out=pacc[:, c:c + 1],
                )
                p16 = wpool.tile([1, 512], BF16, tag="p16", bufs=3, name=f"p16_{b}_{c}")
                nc.scalar.copy(p16[:], p32[:, c * 512:(c + 1) * 512])
                return p16

            def ctx_block(b, c, p16):
                """replicate p across partitions, then per-chunk context on DVE."""
                p32, pacc, ctx_c = state[b]
                rep_ps = misc_ps.tile([128, 512], F32, tag="mps", name=f"rep_ps{b}_{c}")
                nc.tensor.matmul(rep_ps[:], lhsT=ones_sb[:], rhs=p16[:])
                prep = wpool.tile([128, 512], BF16, tag="prep", bufs=3, name=f"prep_{b}_{c}")
                nc.vector.tensor_copy(prep[:], rep_ps[:])
                # ctx_c[c][:, ht] = sum_s keysT[ht][:, s] * p[s] over this chunk
                for ht in range(HT):
                    tout = wpool.tile([128, 512], BF16, tag="ttro", name=f"ttro_{b}_{c}_{ht}")
                    nc.vector.scalar_tensor_tensor(
                        out=tout[:],
                        in0=keys_sb[b][:, (c * HT + ht) * 512: (c * HT + ht) * 512 + 512],
                        scalar=1.0,
                        in1=prep[:],
                        op0=ALU.bypass, op1=ALU.mult,
                        accum_out=ctx_c[c][:, ht:ht + 1],
                    )

            def finalize(b):
                """normalize: weights out, context = (sum_c ctx_c) / sum(p)."""
                p32, pacc, ctx_c = state[b]
                l_t = wpool.tile([1, 1], F32, tag="l", name=f"l_{b}")
                nc.vector.tensor_reduce(l_t[:], pacc[:], axis=mybir.AxisListType.X, op=ALU.add)
                r_t = wpool.tile([1, 1], F32, tag="r", name=f"r_{b}")
                nc.vector.reciprocal(r_t[:], l_t[:])
                w_t = wpool.tile([1, S], F32, tag="w", name=f"w_{b}")
                nc.scalar.mul(w_t[:], p32[:], r_t[:])
                nc.sync.dma_start(out=wout_d[b], in_=w_t[:])
                # replicate r across partitions (fp32 K=1 matmul), scale context
                rep2 = misc_ps.tile([128, 512], F32, tag="mps", name=f"rep2_{b}")
                nc.tensor.matmul(rep2[:, 0:1], lhsT=ones32_sb[:], rhs=r_t[:])
                r128 = wpool.tile([128, 1], F32, tag="r128", name=f"r128_{b}")
                nc.vector.tensor_copy(r128[:], rep2[:, 0:1])
                s01 = wpool.tile([128, HT], F32, tag="s01", name=f"s01_{b}")
                nc.vector.tensor_tensor(out=s01[:], in0=ctx_c[0][:], in1=ctx_c[1][:], op=ALU.add)
                s23 = wpool.tile([128, HT], F32, tag="s23", name=f"s23_{b}")
                nc.vector.tensor_tensor(out=s23[:], in0=ctx_c[2][:], in1=ctx_c[3][:], op=ALU.add)
                csum = wpool.tile([128, HT], F32, tag="csum", name=f"csum_{b}")
                nc.vector.tensor_tensor(out=csum[:], in0=s01[:], in1=s23[:], op=ALU.add)
                ctx = wpool.tile([128, HT], F32, tag="ctx", name=f"ctx_{b}")
                nc.vector.tensor_scalar_mul(ctx[:], csum[:], r128[:])
                nc.sync.dma_start(out=ctxout_d[b], in_=ctx[:])

            # ---- main loop ----
            # The previous chunk's scores / exp / context work is interleaved
            # between this chunk's kp groups so that by the time PE reaches
            # each dependent matmul (scores, replicate) its ACT/DVE inputs are
            # already done — keeps the PE stream stall-free.
            pending = None
            for b in range(BPC):
                state[b] = batch_state(b)
                for c in range(SC):
                    n = b * SC + c + 2
                    if n < BPC * SC:
                        load_k8(n // SC, n % SC)
                    # keysT only feeds the ctx reduction (trails kp by a
                    # chunk), so its chunks trickle in behind the k8 loads:
                    # (b+1, 0..2) during (b, 1..3); (b, 3) during (b, 0).
                    if b + 1 < BPC and c >= 1:
                        if c == 1:
                            keys_sb[b + 1] = kpool.tile(
                                [128, SC * HT * 512], BF16, tag="keys",
                                name=f"keys{b + 1}")
                        nc.sync.dma_start(
                            out=keys_sb[b + 1][:, (c - 1) * HT * 512: c * HT * 512],
                            in_=keysT_d[b + 1, c - 1])
                    if b >= 1 and c == 0:
                        nc.sync.dma_start(
                            out=keys_sb[b][:, 3 * HT * 512: 4 * HT * 512],
                            in_=keysT_d[b, 3])
                    eT = wpool.tile([128, HT * 512], BF16, tag="eT", bufs=3,
                                    name=f"eT_{b}_{c}")
                    for ot in range(HT):
                        if b == 0 and c == 0:
                            qp_group(ot)
                        kp_group(b, c, ot, eT)
                        if pending is not None:
                            pb, pc, peT, psps, pp16 = pending
                            if ot == 0:
                                psps = scores_mms(pb, pc, peT)
                                pending = (pb, pc, peT, psps, pp16)
                            elif ot == 1:
                                pp16 = exp_block(pb, pc, psps)
                                pending = (pb, pc, peT, psps, pp16)
                            elif ot == 3:
                                ctx_block(pb, pc, pp16)
                            elif ot == 5 and pc == SC - 1:
                                finalize(pb)
                    pending = (b, c, eT, None, None)
            pb, pc, peT, _, _ = pending
            # final chunk: scores on PE (the DVE pipeline has no later kp work
            # to hide behind, and the PE is idle here anyway)
            fps = misc_ps.tile([128, 512], F32, tag="mps", name="final_s_ps")
            for ot in range(HT):
                nc.tensor.matmul(
                    fps[0:1, :],
                    lhsT=va16_sb[:, ot:ot + 1],
                    rhs=peT[:, ot * 512:(ot + 1) * 512],
                    start=(ot == 0), stop=(ot == HT - 1),
                )
            p16 = exp_block(pb, pc, fps)
            ctx_block(pb, pc, p16)
            finalize(pb)

    nc.compile()
    return nc


def _prep_inputs(query, keys, Wa_w, Wa_b, Ua_w, Ua_b, Va_w, Va_b):
    bf16 = ml_dtypes.bfloat16
    # Va_b shifts every score equally; softmax is shift-invariant and scores
    # themselves are not returned, so it never affects the output.
    del Va_b
    e4 = ml_dtypes.float8_e4m3fn
    uo = np.ascontiguousarray(
        (Ua_w.T * USCALE).reshape(HT, 128, HT, 128).transpose(2, 1, 0, 3)
    ).reshape(HT, 128, H).astype(np.float32)
    ua8 = uo.astype(e4)
    ua8r = (uo - ua8.astype(np.float32)).astype(e4)
    waT = np.ascontiguousarray(
        Wa_w.T.reshape(HT, 128, HT, 128).transpose(2, 1, 0, 3)).reshape(HT, 128, H).astype(bf16)
    vaT = Va_w[0].reshape(HT, 128).astype(np.float32)
    wbT = np.ascontiguousarray(Wa_b.reshape(HT, 128).T).astype(np.float32)
    ubT = np.ascontiguousarray(Ua_b.reshape(HT, 128).T).astype(np.float32)
    ones = np.ones((1, 128), dtype=bf16)
    ones32 = np.ones((1, 128), dtype=np.float32)

    in_maps = []
    for core in range(N_CORES):
        bs = slice(core * BPC, (core + 1) * BPC)
        kc = np.ascontiguousarray(
            keys[bs].reshape(BPC, SC, 512, HT, 128).transpose(0, 1, 4, 3, 2)
        ).reshape(BPC, SC, 128, HT * 512)
        kT = kc.astype(bf16)
        k16 = (kc * KSCALE).astype(np.float32)
        k8 = k16.astype(e4)
        k8r = (k16 - k8.astype(np.float32)).astype(e4)
        qT = np.ascontiguousarray(
            query[bs, 0, :].T).reshape(HT, 128, BPC).astype(bf16)
        in_maps.append({
            "keysT": kT, "keys8": k8, "keys8r": k8r, "qT": qT, "ua8": ua8,
            "ua8r": ua8r, "waT": waT, "vaT": vaT,
            "wbT": wbT, "ubT": ubT, "ones": ones, "ones32": ones32,
        })
    return in_maps


def kernel(query, keys, Wa_w, Wa_b, Ua_w, Ua_b, Va_w, Va_b):
    if "nc" not in _cache:
        _cache["nc"] = build_module()
    nc = _cache["nc"]

    in_maps = _prep_inputs(query, keys, Wa_w, Wa_b, Ua_w, Ua_b, Va_w, Va_b)
    res = bass_utils.run_bass_kernel_spmd(nc, in_maps, core_ids=list(range(N_CORES)))

    context = np.empty((B, 1, H), dtype=np.float32)
    weights = np.empty((B, 1, S), dtype=np.float32)
    for core in range(N_CORES):
        out = res.results[core]
        weights[core * BPC:(core + 1) * BPC, 0, :] = out["wout"]
        # ctxout[b, p, t] holds context[h = t*128 + p]
        context[core * BPC:(core + 1) * BPC, 0, :] = (
            out["ctxout"].transpose(0, 2, 1).reshape(BPC, H))
    return (context, weights)


# revision 31
# speedup vs baseline: 1.2024x; 1.0044x over previous
"""Bahdanau attention on 8 Trainium2 NeuronCores.

Reference computation (B=32, S=2048, H=1024, fp32):
    q_proj = query @ Wa_w.T + Wa_b            # [B,1,H]
    k_proj = keys @ Ua_w.T + Ua_b             # [B,S,H]
    e      = tanh(q_proj + k_proj)            # [B,S,H]
    scores = e @ Va_w.T + Va_b                # [B,S,1] -> [B,1,S]
    weights = softmax(scores, axis=-1)
    context = weights @ keys                  # [B,1,H]
    returns (context, weights)

Sharding: data-parallel over batch, 4 batches per core; the small
Wa/Ua/Va weights are replicated.

Per-core dataflow (all matmuls contract over the partition dim):
  - keys are uploaded pre-transposed per batch as keysT[h, s] (bf16), so the
    big k_proj matmul runs with Ua_w.T chunks stationary and keysT streaming:
    out[o_tile(128p), s_chunk(512)] accumulates over 8 h-tiles in PSUM.
  - tanh(k_proj + q_proj[o] + Wa_b[o] + Ua_b[o]) is one ScalarE activation per
    tile: the whole per-o bias vector is the per-partition activation bias.
  - scores = Va . e is 8 accumulating matmuls with Va chunk stationary (M=1).
  - softmax skips the max-subtraction (scores are bounded by sum|Va| ~ 16, so
    exp cannot overflow in fp32; softmax is shift-invariant so the result is
    identical to the reference up to rounding).  exp runs on ScalarE with
    accum_out giving the denominator chunks for free.
  - context accumulates per chunk with UNNORMALIZED p = exp(scores): p is
    replicated across partitions by a ones-vector matmul, then
    sum_s p[s] * keysT[h, s] runs on VectorE as fused scalar_tensor_tensor
    ops against the SBUF-resident keysT.  The 1/sum(p) normalization is
    applied once at the end (context scale + weights output), so the context
    reduction overlaps the remaining k_proj matmuls instead of serializing
    after the whole batch.
"""

import numpy as np
import ml_dtypes

import concourse.bass as bass
import concourse.mybir as mybir
import concourse.tile as tile
import concourse.bass_isa as bass_isa
from concourse import bacc
from concourse import bass_utils

BF16 = mybir.dt.bfloat16
F32 = mybir.dt.float32
FP8 = mybir.dt.float8e4
KSCALE = 16.0
USCALE = 256.0
INV_SCALE = 1.0 / (KSCALE * USCALE)
AF = mybir.ActivationFunctionType
ALU = mybir.AluOpType

N_CORES = 8
B, S, H = 32, 2048, 1024
BPC = B // N_CORES          # batches per core
HT = H // 128               # h (and o) tiles of 128
SC = S // 512               # s chunks of 512

_cache = {}


def build_module():
    nc = bacc.Bacc("TRN2", target_bir_lowering=False, debug=False,
                   enable_asserts=False, num_devices=N_CORES)

    # Per-core inputs
    keysT_d = nc.dram_tensor("keysT", [BPC, SC, 128, HT * 512], BF16, kind="ExternalInput").ap()
    qT_d = nc.dram_tensor("qT", [HT, 128, BPC], BF16, kind="ExternalInput").ap()
    keys8_d = nc.dram_tensor("keys8", [BPC, SC, 128, HT * 512], FP8, kind="ExternalInput").ap()
    keys8r_d = nc.dram_tensor("keys8r", [BPC, SC, 128, HT * 512], FP8, kind="ExternalInput").ap()
    # Replicated weights ([ot, p, ht*128+o_in] o-major pieces)
    ua8_d = nc.dram_tensor("ua8", [HT, 128, H], FP8, kind="ExternalInput").ap()
    ua8r_d = nc.dram_tensor("ua8r", [HT, 128, H], FP8, kind="ExternalInput").ap()
    waT_d = nc.dram_tensor("waT", [HT, 128, H], BF16, kind="ExternalInput").ap()
    vaT_d = nc.dram_tensor("vaT", [HT, 128], F32, kind="ExternalInput").ap()
    wbT_d = nc.dram_tensor("wbT", [128, HT], F32, kind="ExternalInput").ap()
    ubT_d = nc.dram_tensor("ubT", [128, HT], F32, kind="ExternalInput").ap()
    ones_d = nc.dram_tensor("ones", [1, 128], BF16, kind="ExternalInput").ap()
    ones32_d = nc.dram_tensor("ones32", [1, 128], F32, kind="ExternalInput").ap()
    # Outputs
    wout_d = nc.dram_tensor("wout", [BPC, S], F32, kind="ExternalOutput").ap()
    ctxout_d = nc.dram_tensor("ctxout", [BPC, 128, HT], F32, kind="ExternalOutput").ap()

    with tile.TileContext(nc) as tc:
        with (
            tc.tile_pool(name="const", bufs=1) as cpool,
            tc.tile_pool(name="keys", bufs=2) as kpool,
            tc.tile_pool(name="work", bufs=2) as wpool,
            tc.tile_pool(name="kp_ps", bufs=6, space="PSUM") as kp_ps,
            tc.tile_pool(name="misc_ps", bufs=2, space="PSUM") as misc_ps,
        ):
            # ---- constants into SBUF ----
            # DMA order is tuned so PE never waits: qp inputs (q, wa piece 0)
            # first, then ua piece 0 + keys0 chunk 0 (first kp group), then the
            # remaining wa/ua pieces interleaved, with keys0 chunks spread in
            # between at the rate the kp groups consume them.
            q_sb = cpool.tile([128, HT * BPC], BF16, tag="q")
            nc.sync.dma_start(out=q_sb[:], in_=qT_d.rearrange("t p b -> p t b"))
            wa_sb = cpool.tile([128, HT * H], BF16, tag="wa")
            nc.sync.dma_start(out=wa_sb[:, 0:H], in_=waT_d[0])
            ua8_sb = cpool.tile([128, HT * H], FP8, tag="ua8")
            nc.sync.dma_start(out=ua8_sb[:, 0:H], in_=ua8_d[0])
            ua8r_sb = cpool.tile([128, HT * H], FP8, tag="ua8r")
            nc.sync.dma_start(out=ua8r_sb[:, 0:H], in_=ua8r_d[0])

            keys_sb = {}
            k8_sb = {}

            def load_k8(b, c):
                t8 = wpool.tile([128, HT * 512], FP8, tag="k8", bufs=4,
                                name=f"k8_{b}_{c}")
                nc.sync.dma_start(out=t8[:], in_=keys8_d[b, c])
                t8r = wpool.tile([128, HT * 512], FP8, tag="k8r", bufs=4,
                                 name=f"k8r_{b}_{c}")
                nc.sync.dma_start(out=t8r[:], in_=keys8r_d[b, c])
                k8_sb[(b, c)] = (t8, t8r)

            load_k8(0, 0)

            va_sb = cpool.tile([128, HT], F32, tag="va")
            nc.sync.dma_start(out=va_sb[:], in_=vaT_d.rearrange("t p -> p t"))
            va16_sb = cpool.tile([128, HT], BF16, tag="va16")
            nc.vector.tensor_copy(va16_sb[:], va_sb[:])
            ones_sb = cpool.tile([1, 128], BF16, tag="ones")
            nc.sync.dma_start(out=ones_sb[:], in_=ones_d)
            ones32_sb = cpool.tile([1, 128], F32, tag="ones32")
            nc.sync.dma_start(out=ones32_sb[:], in_=ones32_d)
            wb_sb = cpool.tile([128, HT], F32, tag="wb")
            nc.sync.dma_start(out=wb_sb[:], in_=wbT_d)
            ub_sb = cpool.tile([128, HT], F32, tag="ub")
            nc.sync.dma_start(out=ub_sb[:], in_=ubT_d)
            bias_sb = cpool.tile([128, HT], F32, tag="bias")
            nc.vector.tensor_tensor(out=bias_sb[:], in0=wb_sb[:], in1=ub_sb[:], op=ALU.add)

            def load_keys(b):
                # chunk-major layout: [:, c*HT*512 + ht*512 + s]
                t = kpool.tile([128, SC * HT * 512], BF16, tag="keys", name=f"keys{b}")
                for c in range(SC):
                    nc.sync.dma_start(
                        out=t[:, c * HT * 512:(c + 1) * HT * 512], in_=keysT_d[b, c])
                keys_sb[b] = t

            for ot in range(1, HT):
                nc.sync.dma_start(out=wa_sb[:, ot * H:(ot + 1) * H], in_=waT_d[ot])
                nc.sync.dma_start(out=ua8_sb[:, ot * H:(ot + 1) * H], in_=ua8_d[ot])
                nc.sync.dma_start(out=ua8r_sb[:, ot * H:(ot + 1) * H], in_=ua8r_d[ot])
                if ot == 3:
                    load_k8(0, 1)
                elif ot == 6:
                    load_k8(0, 2)
            load_keys(0)

            # q_proj group: qp_sb[:, ot*BPC + b] = (Wa_w @ q + Wa_b + Ua_b)[o]
            qp_sb = cpool.tile([128, HT * BPC], F32, tag="qp")

            def qp_group(ot):
                ps = misc_ps.tile([128, 512], F32, tag="mps", name=f"qp_ps{ot}")
                for ht in range(HT):
                    nc.tensor.matmul(
                        ps[:, :BPC],
                        lhsT=wa_sb[:, ot * H + ht * 128: ot * H + ht * 128 + 128],
                        rhs=q_sb[:, ht * BPC:(ht + 1) * BPC],
                        start=(ht == 0), stop=(ht == HT - 1),
                    )
                nc.scalar.activation(
                    qp_sb[:, ot * BPC:(ot + 1) * BPC], ps[:, :BPC],
                    AF.Identity, bias=bias_sb[:, ot:ot + 1],
                )

            # ---- per-batch state ----
            state = {}

            def batch_state(b):
                p32 = wpool.tile([1, S], F32, tag="p32", name=f"p32_{b}")
                pacc = wpool.tile([1, SC], F32, tag="pacc", name=f"pacc_{b}")
                ctx_c = [wpool.tile([128, HT], F32, tag="ctxc", bufs=8, name=f"ctxc_{b}_{c}")
                         for c in range(SC)]
                return p32, pacc, ctx_c

            def kp_group(b, c, ot, eT):
                """one k_proj group: fp8 DoubleRow main + two residual passes
                (all at the same 4096x scale, accumulated in one psum), then
                tanh with the 1/4096 undo folded into the activation scale."""
                ps = kp_ps.tile([128, 512], F32, tag="kp", name=f"kp_ps{b}_{c}_{ot}")
                t8, t8r = k8_sb[(b, c)]
                ua8v = ua8_sb.rearrange("p (t k o) -> p t k o", t=HT, k=HT)
                ua8rv = ua8r_sb.rearrange("p (t k o) -> p t k o", t=HT, k=HT)
                k8v = t8.rearrange("p (k s) -> p k s", k=HT)
                k8rv = t8r.rearrange("p (k s) -> p k s", k=HT)
                passes = [(ua8v, k8v), (ua8v, k8rv), (ua8rv, k8v)]
                n = len(passes) * (HT // 2)
                i = 0
                for uav, kv in passes:
                    for j in range(HT // 2):
                        nc.tensor.matmul(
                            ps[:],
                            lhsT=uav[:, ot, 2 * j:2 * j + 2, :],
                            rhs=kv[:, 2 * j:2 * j + 2, :],
                            start=(i == 0), stop=(i == n - 1),
                            perf_mode=mybir.MatmulPerfMode.DoubleRow,
                        )
                        i += 1
                nc.scalar.activation(
                    eT[:, ot * 512:(ot + 1) * 512], ps[:],
                    AF.Tanh, scale=INV_SCALE,
                    bias=qp_sb[:, ot * BPC + b: ot * BPC + b + 1],
                )

            def scores_mms(b, c, eT):
                """scores = Va . e on DVE: chained (eT[ot] * va[ot]) + acc,
                then one GPSIMD all-reduce across the 128 partitions."""
                acc = None
                for ot in range(HT):
                    nacc = wpool.tile([128, 512], F32, tag="sacc", bufs=3,
                                      name=f"sacc_{b}_{c}_{ot}")
                    if acc is None:
                        nc.vector.tensor_scalar_mul(
                            nacc[:], eT[:, ot * 512:(ot + 1) * 512], va_sb[:, ot:ot + 1])
                    else:
                        nc.vector.scalar_tensor_tensor(
                            out=nacc[:], in0=eT[:, ot * 512:(ot + 1) * 512],
                            scalar=va_sb[:, ot:ot + 1], in1=acc[:],
                            op0=ALU.mult, op1=ALU.add)
                    acc = nacc
                sred = wpool.tile([128, 512], F32, tag="sred", bufs=2,
                                  name=f"sred_{b}_{c}")
                nc.gpsimd.partition_all_reduce(sred[:], acc[:], channels=128,
                                               reduce_op=bass_isa.ReduceOp.add)
                return sred

            def exp_block(b, c, sps):
                """p = exp(scores) chunk + running denominator + bf16 copy."""
                p32, pacc, ctx_c = state[b]
                nc.scalar.activation(
                    p32[:, c * 512:(c + 1) * 512], sps[0:1, :],
                    AF.Exp, accum_out=pacc[:, c:c + 1],
                )
                p16 = wpool.tile([1, 512], BF16, tag="p16", bufs=3, name=f"p16_{b}_{c}")
                nc.scalar.copy(p16[:], p32[:, c * 512:(c + 1) * 512])
                return p16

            def ctx_block(b, c, p16):
                """replicate p across partitions, then per-chunk context on DVE."""
                p32, pacc, ctx_c = state[b]
                rep_ps = misc_ps.tile([128, 512], F32, tag="mps", name=f"rep_ps{b}_{c}")
                nc.tensor.matmul(rep_ps[:], lhsT=ones_sb[:], rhs=p16[:])
                prep = wpool.tile([128, 512], BF16, tag="prep", bufs=3, name=f"prep_{b}_{c}")
                nc.vector.tensor_copy(prep[:], rep_ps[:])
                # ctx_c[c][:, ht] = sum_s keysT[ht][:, s] * p[s] over this chunk
                for ht in range(HT):
                    tout = wpool.tile([128, 512], BF16, tag="ttro", name=f"ttro_{b}_{c}_{ht}")
                    nc.vector.scalar_tensor_tensor(
                        out=tout[:],
                        in0=keys_sb[b][:, (c * HT + ht) * 512: (c * HT + ht) * 512 + 512],
                        scalar=1.0,
                        in1=prep[:],
                        op0=ALU.bypass, op1=ALU.mult,
                        accum_out=ctx_c[c][:, ht:ht + 1],
                    )

            def finalize(b):
                """normalize: weights out, context = (sum_c ctx_c) / sum(p)."""
                p32, pacc, ctx_c = state[b]
                l_t = wpool.tile([1, 1], F32, tag="l", name=f"l_{b}")
                nc.vector.tensor_reduce(l_t[:], pacc[:], axis=mybir.AxisListType.X, op=ALU.add)
                r_t = wpool.tile([1, 1], F32, tag="r", name=f"r_{b}")
                nc.vector.reciprocal(r_t[:], l_t[:])
                w_t = wpool.tile([1, S], F32, tag="w", name=f"w_{b}")
                nc.scalar.mul(w_t[:], p32[:], r_t[:])
                nc.sync.dma_start(out=wout_d[b], in_=w_t[:])
                # replicate r across partitions (fp32 K=1 matmul), scale context
                rep2 = misc_ps.tile([128, 512], F32, tag="mps", name=f"rep2_{b}")
                nc.tensor.matmul(rep2[:, 0:1], lhsT=ones32_sb[:], rhs=r_t[:])
                r128 = wpool.tile([128, 1], F32, tag="r128", name=f"r128_{b}")
                nc.vector.tensor_copy(r128[:], rep2[:, 0:1])
                s01 = wpool.tile([128, HT], F32, tag="s01", name=f"s01_{b}")
                nc.vector.tensor_tensor(out=s01[:], in0=ctx_c[0][:], in1=ctx_c[1][:], op=ALU.add)
                s23 = wpool.tile([128, HT], F32, tag="s23", name=f"s23_{b}")
                nc.vector.tensor_tensor(out=s23[:], in0=ctx_c[2][:], in1=ctx_c[3][:], op=ALU.add)
                csum = wpool.tile([128, HT], F32, tag="csum", name=f"csum_{b}")
                nc.vector.tensor_tensor(out=csum[:], in0=s01[:], in1=s23[:], op=ALU.add)
                ctx = wpool.tile([128, HT], F32, tag="ctx", name=f"ctx_{b}")
                nc.vector.tensor_scalar_mul(ctx[:], csum[:], r128[:])
                nc.sync.dma_start(out=ctxout_d[b], in_=ctx[:])

            # ---- main loop ----
            # The previous chunk's scores / exp / context work is interleaved
            # between this chunk's kp groups so that by the time PE reaches
            # each dependent matmul (scores, replicate) its ACT/DVE inputs are
            # already done — keeps the PE stream stall-free.
            pending = None
            for b in range(BPC):
                state[b] = batch_state(b)
                for c in range(SC):
                    n = b * SC + c + 3
                    if n < BPC * SC:
                        load_k8(n // SC, n % SC)
                    # keysT only feeds the ctx reduction (trails kp by a
                    # chunk), so its chunks trickle in behind the k8 loads:
                    # (b+1, 0..2) during (b, 1..3); (b, 3) during (b, 0).
                    if b + 1 < BPC and c >= 1:
                        if c == 1:
                            keys_sb[b + 1] = kpool.tile(
                                [128, SC * HT * 512], BF16, tag="keys",
                                name=f"keys{b + 1}")
                        nc.sync.dma_start(
                            out=keys_sb[b + 1][:, (c - 1) * HT * 512: c * HT * 512],
                            in_=keysT_d[b + 1, c - 1])
                    if b >= 1 and c == 0:
                        nc.sync.dma_start(
                            out=keys_sb[b][:, 3 * HT * 512: 4 * HT * 512],
                            in_=keysT_d[b, 3])
                    eT = wpool.tile([128, HT * 512], BF16, tag="eT", bufs=3,
                                    name=f"eT_{b}_{c}")
                    for ot in range(HT):
                        if b == 0 and c == 0:
                            qp_group(ot)
                        kp_group(b, c, ot, eT)
                        if pending is not None:
                            pb, pc, peT, psps, pp16 = pending
                            if ot == 0:
                                psps = scores_mms(pb, pc, peT)
                                pending = (pb, pc, peT, psps, pp16)
                            elif ot == 1:
                                pp16 = exp_block(pb, pc, psps)
                                pending = (pb, pc, peT, psps, pp16)
                            elif ot == 3:
                                ctx_block(pb, pc, pp16)
                            elif ot == 5 and pc == SC - 1:
                                finalize(pb)
                    pending = (b, c, eT, None, None)
            pb, pc, peT, _, _ = pending
            # final chunk: scores on PE (the DVE pipeline has no later kp work
            # to hide behind, and the PE is idle here anyway)
            fps = misc_ps.tile([128, 512], F32, tag="mps", name="final_s_ps")
            for ot in range(HT):
                nc.tensor.matmul(
                    fps[0:1, :],
                    lhsT=va16_sb[:, ot:ot + 1],
                    rhs=peT[:, ot * 512:(ot + 1) * 512],
                    start=(ot == 0), stop=(ot == HT - 1),
                )
            p16 = exp_block(pb, pc, fps)
            ctx_block(pb, pc, p16)
            finalize(pb)

    nc.compile()
    return nc


def _prep_inputs(query, keys, Wa_w, Wa_b, Ua_w, Ua_b, Va_w, Va_b):
    bf16 = ml_dtypes.bfloat16
    # Va_b shifts every score equally; softmax is shift-invariant and scores
    # themselves are not returned, so it never affects the output.
    del Va_b
    e4 = ml_dtypes.float8_e4m3fn
    uo = np.ascontiguousarray(
        (Ua_w.T * USCALE).reshape(HT, 128, HT, 128).transpose(2, 1, 0, 3)
    ).reshape(HT, 128, H).astype(np.float32)
    ua8 = uo.astype(e4)
    ua8r = (uo - ua8.astype(np.float32)).astype(e4)
    waT = np.ascontiguousarray(
        Wa_w.T.reshape(HT, 128, HT, 128).transpose(2, 1, 0, 3)).reshape(HT, 128, H).astype(bf16)
    vaT = Va_w[0].reshape(HT, 128).astype(np.float32)
    wbT = np.ascontiguousarray(Wa_b.reshape(HT, 128).T).astype(np.float32)
    ubT = np.ascontiguousarray(Ua_b.reshape(HT, 128).T).astype(np.float32)
    ones = np.ones((1, 128), dtype=bf16)
    ones32 = np.ones((1, 128), dtype=np.float32)

    in_maps = []
    for core in range(N_CORES):
        bs = slice(core * BPC, (core + 1) * BPC)
        kc = np.ascontiguousarray(
            keys[bs].reshape(BPC, SC, 512, HT, 128).transpose(0, 1, 4, 3, 2)
        ).reshape(BPC, SC, 128, HT * 512)
        kT = kc.astype(bf16)
        k16 = (kc * KSCALE).astype(np.float32)
        k8 = k16.astype(e4)
        k8r = (k16 - k8.astype(np.float32)).astype(e4)
        qT = np.ascontiguousarray(
            query[bs, 0, :].T).reshape(HT, 128, BPC).astype(bf16)
        in_maps.append({
            "keysT": kT, "keys8": k8, "keys8r": k8r, "qT": qT, "ua8": ua8,
            "ua8r": ua8r, "waT": waT, "vaT": vaT,
            "wbT": wbT, "ubT": ubT, "ones": ones, "ones32": ones32,
        })
    return in_maps


def kernel(query, keys, Wa_w, Wa_b, Ua_w, Ua_b, Va_w, Va_b):
    if "nc" not in _cache:
        _cache["nc"] = build_module()
    nc = _cache["nc"]

    in_maps = _prep_inputs(query, keys, Wa_w, Wa_b, Ua_w, Ua_b, Va_w, Va_b)
    res = bass_utils.run_bass_kernel_spmd(nc, in_maps, core_ids=list(range(N_CORES)))

    context = np.empty((B, 1, H), dtype=np.float32)
    weights = np.empty((B, 1, S), dtype=np.float32)
    for core in range(N_CORES):
        out = res.results[core]
        weights[core * BPC:(core + 1) * BPC, 0, :] = out["wout"]
        # ctxout[b, p, t] holds context[h = t*128 + p]
        context[core * BPC:(core + 1) * BPC, 0, :] = (
            out["ctxout"].transpose(0, 2, 1).reshape(BPC, H))
    return (context, weights)
